# revision 14
# baseline (speedup 1.0000x reference)
"""Chopfield attention (complex QKV projections + real-part softmax attention)
on 8 Trainium2 NeuronCores.

Math (reference):
    Q = R @ W_Q ; K = Y @ W_K ; V = Y @ W_V          (complex, [4096,1024])
    Z = BETA * Re(conj(Q) @ K^T)                      [4096,4096] real
      = (BETA*Q_re) @ K_re^T + (BETA*Q_im) @ K_im^T
    A = softmax(Z, axis=-1)                           real
    out = A @ V                                       (complex)

Sharding: queries (R rows) and keys (Y rows) are both sharded 8-way.
Each core computes its K/V shard, AllGathers K^T and V, computes local
Q, scores, softmax and A@V for its 512 query rows.

Precision: the softmax is near-one-hot (score std ~2900), so the whole
Q/K score chain must be fp32-accurate. All score-chain matmuls use a
3-pass fp16 hi/lo split (fp16 products are exact on the PE and
accumulate in fp32), which lands within ~2e-3 of a pure-fp32 pipeline.
The V path tolerates fp16 single-pass.
"""

import numpy as np

import concourse.bacc as bacc
import concourse.mybir as mybir
import concourse.tile as tile
from concourse.bass_utils import run_bass_kernel_spmd

BETA = 0.03125
P = 128
FP16 = mybir.dt.float16
FP32 = mybir.dt.float32
X = mybir.AxisListType.X


class Cfg:
    def __init__(self, N=4096, M=4096, D=1024, NC=8):
        self.N, self.M, self.D, self.NC = N, M, D, NC
        self.NL = N // NC          # local query rows
        self.ML = M // NC          # local key rows
        self.DT = D // P           # contraction tiles
        self.QTS = self.NL // P    # local query partition-tiles
        self.MTS = self.ML // P    # local key partition-tiles
        self.DF = min(512, D)      # free-dim chunk for D-wide outputs
        self.DCH = D // self.DF    # chunks of D
        self.MTG = M // P          # global key partition-tiles
        self.KHALF = 2 if self.DT % 2 == 0 else 1   # score K-stream halves
        self.SLOT = D * self.ML    # elements per gathered tensor slot
        # slots: 0..3 = KT(re_h, re_l, im_h, im_l) [D, ML]; 4,5 = V(re, im) [ML, D]
        self.NSLOT = 6


def build(cfg: Cfg, reps: int = 1, no_collective: bool = False, stop_after: str | None = None):
    c = cfg
    nc = bacc.Bacc("TRN2", target_bir_lowering=False, debug=False, num_devices=c.NC)

    def din(name, shape, dt=FP16):
        return nc.dram_tensor(name, shape, dt, kind="ExternalInput")

    # stationary weights [D, D] (column-sliced per output tile at load time)
    # weights host-swizzled to [out_block, partition, in_tile*cols] so every
    # per-output-tile slice is one fully-contiguous DMA
    wq = {}
    for comp in ("re", "im", "s"):
        for lvl in ("h", "l"):
            wq[comp, lvl] = din(f"wq_{comp}_{lvl}", [c.DT, P, c.DT * P])
    wk = {}
    for comp in ("re", "im", "s"):
        for lvl in ("h", "l"):
            wk[comp, lvl] = din(f"wk_{comp}_{lvl}", [c.DT, P, c.DT * P])
    wv = {n: din(f"wv_{n}", [c.DCH, P, c.DT * c.DF]) for n in ("re", "im", "s")}

    # moving operands: R^T and Y^T with hi/lo splits (+re+im sum variants
    # for the Karatsuba complex-product decomposition)
    rt = {}
    yt = {}
    for comp in ("re", "im", "s"):
        for lvl in ("h", "l"):
            rt[comp, lvl] = din(f"rt_{comp}_{lvl}", [P, c.DT * c.NL])
            yt[comp, lvl] = din(f"yt_{comp}_{lvl}", [P, c.DT * c.ML])

    ident = din("ident", [P, P])

    o_re = nc.dram_tensor("o_re", [c.NL, c.D], FP32, kind="ExternalOutput")
    o_im = nc.dram_tensor("o_im", [c.NL, c.D], FP32, kind="ExternalOutput")

    with tile.TileContext(nc) as tc:
        with (
            tc.tile_pool(name="pers", bufs=1) as pers,
            tc.tile_pool(name="ps", bufs=1, space="PSUM") as ps,
            tc.tile_pool(name="dram", bufs=1, space="DRAM") as dram,
        ):
            def emit(rep):
                prp = tc.alloc_tile_pool(name=f"prp{rep}", bufs=1)
                qrt = tc.alloc_tile_pool(name=f"qrt{rep}", bufs=1)
                kvp = tc.alloc_tile_pool(name=f"kvp{rep}", bufs=1)
                # ---------- load Y^T (moving operand for K and V projections)
                _scope_stack = []

                def scope(name):
                    if _scope_stack:
                        pn, pid = _scope_stack.pop()
                        nc.leave_named_scope(pn, pid)
                    if name is not None:
                        sid, _ = nc.enter_named_scope(name)
                        _scope_stack.append((name, sid))

                scope("yload")
                yts = {}
                for key, t in yt.items():
                    yts[key] = kvp.tile([P, c.DT * c.ML], FP16, tag=f"yt{key}", name=f"yt_{key[0]}_{key[1]}_{rep}")
                    nc.scalar.dma_start(yts[key][:], t.ap())

                ident_sb = pers.tile([P, P], FP16, tag="ident")
                nc.sync.dma_start(ident_sb[:], ident.ap())

                # collective bounce buffers (flat fp16 element counts)
                agk_in = dram.tile([4 * c.SLOT], FP16)
                agk_out = dram.tile([c.NC * 4 * c.SLOT], FP16, addr_space="Shared")
                agv_in = dram.tile([2 * c.SLOT], FP16)
                agv_out = dram.tile([c.NC * 2 * c.SLOT], FP16, addr_space="Shared")

                # ---------- K^T projection: K^T = W_K^T @ Y^T  (3-pass split)
                # K_re^T = WKre^T@YTre + WKim^T@YTimn ; K_im^T = WKim^T@YTre + WKre^T@YTim

                def proj_qk(w, mov, wtag, mw, pool, out_sb=None, bounce_si=None):
                    """Karatsuba complex projection, 3-pass fp16 split per product.
                    m1 = A_re@W_re, m2 = A_im@W_im, m3 = (A_re+A_im)@(W_re+W_im);
                    out_re = m1 - m2, out_im = m3 - m1 - m2."""
                    for dt_out in range(c.DT):
                        wsl = pool.tile([P, 6 * c.DT * P], FP16, tag=wtag, bufs=2)
                        widx = {("re", "h"): 0, ("re", "l"): 1, ("im", "h"): 2,
                                ("im", "l"): 3, ("s", "h"): 4, ("s", "l"): 5}
                        for (wc, wl), wi in widx.items():
                            nc.sync.dma_start(
                                wsl[:, wi * c.DT * P : (wi + 1) * c.DT * P],
                                w[wc, wl].ap()[dt_out],
                            )

                        def wslice(wc, wl, ki):
                            wi = widx[wc, wl]
                            return wsl[:, wi * c.DT * P + ki * P : wi * c.DT * P + (ki + 1) * P]

                        m = {}
                        for prod, (wc, mc) in enumerate(
                            [("re", "re"), ("im", "im"), ("s", "s")]
                        ):
                            pt = ps.tile([P, 512], FP32, tag="ps", bufs=6)
                            m[prod] = pt[:, :mw]
                            nmm = c.DT * 3
                            i = 0
                            for ki in range(c.DT):
                                for wl, ml in (("h", "h"), ("h", "l"), ("l", "h")):
                                    nc.tensor.matmul(
                                        m[prod],
                                        wslice(wc, wl, ki),
                                        mov[mc, ml][:, ki * mw : ki * mw + mw],
                                        start=(i == 0),
                                        stop=(i == nmm - 1),
                                    )
                                    i += 1
                        # DVE may read only ONE operand from PSUM per inst:
                        # stage m2 in SBUF, then chain single-PSUM subtracts.
                        m2s = pool.tile([P, 512], FP32, tag=wtag + "m2s", bufs=2)
                        nc.vector.tensor_copy(m2s[:, :mw], m[1])
                        dre = pool.tile([P, 512], FP32, tag=wtag + "dre", bufs=2)
                        nc.vector.tensor_sub(dre[:, :mw], m[0], m2s[:, :mw])
                        dim = pool.tile([P, 512], FP32, tag=wtag + "dim", bufs=2)
                        nc.vector.tensor_sub(dim[:, :mw], m[2], m2s[:, :mw])
                        nc.vector.tensor_sub(dim[:, :mw], dim[:, :mw], m[0])
                        for comp, d in (("re", dre), ("im", dim)):
                            if out_sb is not None:
                                hi = out_sb[comp, "h"][:, dt_out * mw : (dt_out + 1) * mw]
                                lo = out_sb[comp, "l"][:, dt_out * mw : (dt_out + 1) * mw]
                            else:
                                hi = pool.tile([P, mw], FP16, tag=wtag + comp + "hi", bufs=2)
                                lo = pool.tile([P, mw], FP16, tag=wtag + comp + "lo", bufs=2)
                            nc.vector.tensor_copy(hi if out_sb is not None else hi[:], d[:, :mw])
                            nc.vector.tensor_sub(lo if out_sb is not None else lo[:], d[:, :mw], hi if out_sb is not None else hi[:])
                            if out_sb is None:
                                for lvl, t in (("h", hi), ("l", lo)):
                                    si = bounce_si[comp, lvl]
                                    dst = agk_in[
                                        si * c.SLOT + dt_out * P * mw : si * c.SLOT + (dt_out + 1) * P * mw
                                    ].rearrange("(p m) -> p m", p=P)
                                    nc.gpsimd.dma_start(dst, t[:])

                kp = tc.alloc_tile_pool(name=f"kp{rep}", bufs=1)
                scope("kproj")
                proj_qk(
                    wk, yts, "wksl", c.ML, kp,
                    bounce_si={("re", "h"): 0, ("re", "l"): 1, ("im", "h"): 2, ("im", "l"): 3},
                )
                kp.release()
                scope("agk")
                if not no_collective:
                    nc.gpsimd.collective_compute(
                        "AllGather",
                        mybir.AluOpType.bypass,
                        replica_groups=[list(range(c.NC))],
                        ins=[agk_in.opt()],
                        outs=[agk_out.opt()],
                    )

                # R^T loads hoisted here: the scalar DMA ring is idle during the
                # V projection, so Q's moving operands are resident before Q starts.
                scope("rload")
                rts = {}
                for key, t in rt.items():
                    rts[key] = qrt.tile([P, c.DT * c.NL], FP16, tag=f"rt{key}", name=f"rt_{key[0]}_{key[1]}_{rep}")
                    nc.scalar.dma_start(rts[key][:], t.ap())

                vp = tc.alloc_tile_pool(name=f"vp{rep}", bufs=1)
                scope("vproj")
                # ---------- V projection (single-pass fp16, Karatsuba):
                # m1 = Yre@WVre, m2 = Yim@WVim, m3 = (Yre+Yim)@(WVre+WVim)
                # V_re = m1 - m2 ; V_im = m3 - m1 - m2
                for dch in range(c.DCH):
                    wvsl = vp.tile([P, 3 * c.DT * c.DF], FP16, tag="wvsl", bufs=1)
                    wvidx = {"re": 0, "im": 1, "s": 2}
                    for wn, wi in wvidx.items():
                        nc.sync.dma_start(
                            wvsl[:, wi * c.DT * c.DF : (wi + 1) * c.DT * c.DF],
                            wv[wn].ap()[dch],
                        )
                    for mt in range(c.MTS):
                        m = {}
                        for prod, yc in enumerate(("re", "im", "s")):
                            pt = ps.tile([P, 512], FP32, tag="ps", bufs=6)
                            m[prod] = pt[:, : c.DF]
                            wn = yc
                            for ki in range(c.DT):
                                nc.tensor.matmul(
                                    m[prod],
                                    yts[yc, "h"][:, ki * c.ML + mt * P : ki * c.ML + (mt + 1) * P],
                                    wvsl[:, wvidx[wn] * c.DT * c.DF + ki * c.DF : wvidx[wn] * c.DT * c.DF + (ki + 1) * c.DF],
                                    start=(ki == 0),
                                    stop=(ki == c.DT - 1),
                                )
                        vm2s = vp.tile([P, c.DF], FP32, tag="vm2s", bufs=2)
                        nc.vector.tensor_copy(vm2s[:], m[1])
                        for comp, si in (("re", 0), ("im", 1)):
                            vout = vp.tile([P, c.DF], FP16, tag="vout", bufs=4)
                            if comp == "re":
                                nc.vector.tensor_sub(vout[:], m[0], vm2s[:])
                            else:
                                vim1 = vp.tile([P, c.DF], FP32, tag="vim1", bufs=2)
                                nc.vector.tensor_sub(vim1[:], m[2], vm2s[:])
                                nc.vector.tensor_sub(vout[:], vim1[:], m[0])
                            dst = agv_in[si * c.SLOT : (si + 1) * c.SLOT].rearrange(
                                "(m p dc d) -> m p dc d", m=c.MTS, p=P, dc=c.DCH
                            )[mt, :, dch, :]
                            nc.gpsimd.dma_start(dst, vout[:])

                # ---------- AllGather V (A@V consumes it much later)
                scope("agv")
                if not no_collective:
                    nc.gpsimd.collective_compute(
                        "AllGather",
                        mybir.AluOpType.bypass,
                        replica_groups=[list(range(c.NC))],
                        ins=[agv_in.opt()],
                        outs=[agv_out.opt()],
                    )
                if stop_after == "vproj":
                    vp.release()
                    kvp.release()
                    qrt.release()
                    prp.release()
                    scope(None)
                    return

                vp.release()
                kvp.release()

                # ---------- Q^T projection (R^T loads hoisted above, before V proj)
                scope("qproj")
                qp = tc.alloc_tile_pool(name=f"qp{rep}", bufs=1)
                qt_sb = {}
                for comp in ("re", "im"):
                    for lvl in ("h", "l"):
                        qt_sb[comp, lvl] = prp.tile([P, c.DT * c.NL], FP16, tag=f"qt{comp}{lvl}", name=f"qt_{comp}_{lvl}_{rep}")
                proj_qk(wq, rts, "wqsl", c.NL, qp, out_sb=qt_sb)
                qp.release()
                qrt.release()
                if stop_after == "qproj":
                    prp.release()
                    scope(None)
                    return

                # ---------- scores + streaming softmax (per key-shard chunk)
                # Z[q, m] = QT^T @ KT ; chunk max -> exp(Z - cmax); rescale later.
                scope("scores")
                scp = tc.alloc_tile_pool(name=f"scp{rep}", bufs=1)
                p_sb = [prp.tile([P, c.M], FP16, tag=f"p{qt}", name=f"p_{qt}_{rep}") for qt in range(c.QTS)]
                cm = [prp.tile([P, c.NC], FP32, tag=f"cm{qt}", name=f"cm_{qt}_{rep}") for qt in range(c.QTS)]
                ncm = [prp.tile([P, c.NC], FP32, tag=f"ncm{qt}", name=f"ncm_{qt}_{rep}") for qt in range(c.QTS)]

                kdh = c.DT // c.KHALF  # d-tiles per streamed half
                for r in range(c.NC):
                    halves = []
                    for h in range(c.KHALF):
                        ktl = scp.tile([P, 4 * kdh * c.ML], FP16, tag="ktl", bufs=3)
                        for si in range(4):
                            src = agk_out[
                                r * 4 * c.SLOT
                                + si * c.SLOT
                                + h * kdh * P * c.ML : r * 4 * c.SLOT
                                + si * c.SLOT
                                + (h + 1) * kdh * P * c.ML
                            ].rearrange("(t p m) -> p t m", p=P, m=c.ML)
                            nc.scalar.dma_start(
                                ktl[:, si * kdh * c.ML : (si + 1) * kdh * c.ML].rearrange(
                                    "p (t m) -> p t m", m=c.ML
                                ),
                                src,
                            )
                        halves.append(ktl)

                    def ktslice(comp, lvl, ki):
                        si = {("re", "h"): 0, ("re", "l"): 1, ("im", "h"): 2, ("im", "l"): 3}[comp, lvl]
                        t = halves[ki // kdh]
                        k = ki % kdh
                        return t[:, si * kdh * c.ML + k * c.ML : si * kdh * c.ML + (k + 1) * c.ML]

                    for qt in range(c.QTS):
                        zp = ps.tile([P, 512], FP32, tag="ps", bufs=6)
                        zacc = zp[:, : c.ML]
                        nmm = 2 * c.DT * 3
                        i = 0
                        for comp in ("re", "im"):
                            for ki in range(c.DT):
                                for ql, kl in (("h", "h"), ("h", "l"), ("l", "h")):
                                    nc.tensor.matmul(
                                        zacc,
                                        qt_sb[comp, ql][:, ki * c.NL + qt * P : ki * c.NL + (qt + 1) * P],
                                        ktslice(comp, kl, ki),
                                        start=(i == 0),
                                        stop=(i == nmm - 1),
                                    )
                                    i += 1
                        nc.vector.reduce_max(cm[qt][:, r : r + 1], zacc, axis=X)
                        nc.vector.tensor_scalar_mul(
                            ncm[qt][:, r : r + 1], cm[qt][:, r : r + 1], -1.0
                        )
                        nc.scalar.activation(
                            p_sb[qt][:, r * c.ML : (r + 1) * c.ML],
                            zacc,
                            mybir.ActivationFunctionType.Exp,
                            bias=ncm[qt][:, r : r + 1],
                            scale=1.0,
                        )

                # ---------- finalize softmax: rescale chunks to the global max
                scope("smax")
                recip = []
                for qt in range(c.QTS):
                    ngm = prp.tile([P, 1], FP32, tag=f"ngm{qt}")
                    nc.vector.tensor_reduce(
                        ngm[:], ncm[qt][:], op=mybir.AluOpType.min, axis=X
                    )
                    fac = prp.tile([P, c.NC], FP32, tag=f"fac{qt}")
                    nc.scalar.activation(
                        fac[:],
                        ncm[qt][:],
                        mybir.ActivationFunctionType.Exp,
                        bias=ngm[:, 0:1],
                        scale=-1.0,
                    )
                    for r in range(c.NC):
                        nc.vector.tensor_scalar_mul(
                            p_sb[qt][:, r * c.ML : (r + 1) * c.ML],
                            p_sb[qt][:, r * c.ML : (r + 1) * c.ML],
                            fac[:, r : r + 1],
                        )
                    ssum = prp.tile([P, 1], FP32, tag=f"ssum{qt}")
                    nc.vector.reduce_sum(ssum[:], p_sb[qt][:], axis=X)
                    rc = prp.tile([P, 1], FP32, tag=f"rcp{qt}")
                    nc.vector.reciprocal(rc[:], ssum[:])
                    recip.append(rc)

                scp.release()
                if stop_after == "scores":
                    prp.release()
                    scope(None)
                    return

                # ---------- transpose P -> P^T tiles ([m-part, q-free])
                scope("transp")
                avp = tc.alloc_tile_pool(name=f"avp{rep}", bufs=1)
                pt_sb = [avp.tile([P, c.NL], FP16, tag=f"pt{mtg}", name=f"pt_{mtg}_{rep}") for mtg in range(c.MTG)]
                for mtg in range(c.MTG):
                    tp = ps.tile([P, 512], FP16, tag="dsc", bufs=2)
                    tacc = tp[:, : c.NL]
                    for qt in range(c.QTS):
                        nc.tensor.matmul(
                            tacc[:, qt * P : (qt + 1) * P],
                            p_sb[qt][:, mtg * P : (mtg + 1) * P],
                            ident_sb[:],
                            start=True,
                            stop=True,
                            is_transpose=True,
                        )
                    nc.vector.tensor_copy(pt_sb[mtg][:], tacc)
                if stop_after == "transp":
                    avp.release()
                    prp.release()
                    scope(None)
                    return

                # ---------- A @ V (+ 1/sum scaling)
                scope("av")
                for comp, odram in (("re", o_re), ("im", o_im)):
                    si = 0 if comp == "re" else 1
                    for dch in range(c.DCH):
                        vh = avp.tile([P, c.MTG * c.DF], FP16, tag="vh", bufs=2)
                        for r in range(c.NC):
                            src = agv_out[
                                r * 2 * c.SLOT + si * c.SLOT : r * 2 * c.SLOT + (si + 1) * c.SLOT
                            ].rearrange("(m p dc d) -> dc p m d", m=c.MTS, p=P, dc=c.DCH)[dch]
                            nc.sync.dma_start(
                                vh[
                                    :, r * c.MTS * c.DF : (r + 1) * c.MTS * c.DF
                                ].rearrange("p (m d) -> p m d", m=c.MTS),
                                src,
                            )
                        for qt in range(c.QTS):
                            op_ = ps.tile([P, 512], FP32, tag="ps", bufs=6)
                            oacc = op_[:, : c.DF]
                            for mtg in range(c.MTG):
                                nc.tensor.matmul(
                                    oacc,
                                    pt_sb[mtg][:, qt * P : (qt + 1) * P],
                                    vh[:, mtg * c.DF : (mtg + 1) * c.DF],
                                    start=(mtg == 0),
                                    stop=(mtg == c.MTG - 1),
                                )
                            osb = avp.tile([P, c.DF], FP32, tag="osb", bufs=4)
                            nc.vector.tensor_scalar_mul(osb[:], oacc, recip[qt][:, 0:1])
                            nc.sync.dma_start(
                                odram.ap()[
                                    qt * P : (qt + 1) * P, dch * c.DF : (dch + 1) * c.DF
                                ],
                                osb[:],
                            )
                avp.release()
                prp.release()
                scope(None)

            for rep in range(reps):
                emit(rep)

    nc.compile()
    return nc


def _split16(x):
    h = x.astype(np.float16)
    l = (x - h.astype(np.float32)).astype(np.float16)
    return h, l


def prep_inputs(cfg, R_re, R_im, Y_re, Y_im, W_Q_re, W_Q_im, W_K_re, W_K_im, W_V_re, W_V_im):
    """Host-side sharding + fp16 hi/lo split + transposes. Returns in_maps."""
    c = cfg
    f32 = np.float32
    wq_re = np.ascontiguousarray(W_Q_re, dtype=f32) * BETA
    wq_im = np.ascontiguousarray(W_Q_im, dtype=f32) * BETA
    wk_re = np.ascontiguousarray(W_K_re, dtype=f32)
    wk_im = np.ascontiguousarray(W_K_im, dtype=f32)
    wv_re = np.ascontiguousarray(W_V_re, dtype=f32)
    wv_im = np.ascontiguousarray(W_V_im, dtype=f32)
    wqs = {"re": _split16(wq_re), "im": _split16(wq_im), "s": _split16(wq_re + wq_im)}
    wks = {"re": _split16(wk_re), "im": _split16(wk_im), "s": _split16(wk_re + wk_im)}
    ident = np.eye(P, dtype=np.float16)

    DT, DCH, DF = cfg.DT, cfg.DCH, cfg.DF

    def _wsw(w16, ocols):
        # [d_in, d_out] -> [d_out_block, p, d_in_tile * ocols], contiguous
        ob = w16.shape[1] // ocols
        return np.ascontiguousarray(
            w16.reshape(DT, P, ob, ocols).transpose(2, 1, 0, 3).reshape(ob, P, DT * ocols)
        )

    shared = {}
    for comp in ("re", "im", "s"):
        for li, lvl in enumerate(("h", "l")):
            shared[f"wq_{comp}_{lvl}"] = _wsw(wqs[comp][li], P)
            shared[f"wk_{comp}_{lvl}"] = _wsw(wks[comp][li], P)
    shared["wv_re"] = _wsw(wv_re.astype(np.float16), DF)
    shared["wv_im"] = _wsw(wv_im.astype(np.float16), DF)
    shared["wv_s"] = _wsw((wv_re + wv_im).astype(np.float16), DF)
    shared["ident"] = ident

    in_maps = []
    for r in range(c.NC):
        m = dict(shared)
        rsl = slice(r * c.NL, (r + 1) * c.NL)
        ysl = slice(r * c.ML, (r + 1) * c.ML)
        rre_t = np.ascontiguousarray(np.asarray(R_re[rsl], dtype=f32).T)
        rim_t = np.ascontiguousarray(np.asarray(R_im[rsl], dtype=f32).T)
        yre_t = np.ascontiguousarray(np.asarray(Y_re[ysl], dtype=f32).T)
        yim_t = np.ascontiguousarray(np.asarray(Y_im[ysl], dtype=f32).T)
        for base, arr in (("rt_re", rre_t), ("rt_im", rim_t), ("rt_s", rre_t + rim_t),
                          ("yt_re", yre_t), ("yt_im", yim_t), ("yt_s", yre_t + yim_t)):
            h, l = _split16(arr)
            mw = arr.shape[1]
            for lvl, a in (("h", h), ("l", l)):
                m[f"{base}_{lvl}"] = np.ascontiguousarray(
                    a.reshape(DT, P, mw).transpose(1, 0, 2).reshape(P, DT * mw)
                )
        in_maps.append(m)
    return in_maps


_NC_CACHE = {}


def kernel(**inputs) -> np.ndarray:
    cfg = Cfg()
    if "full" not in _NC_CACHE:
        _NC_CACHE["full"] = build(cfg, 1)
    nc = _NC_CACHE["full"]
    in_maps = prep_inputs(cfg, **inputs)
    res = run_bass_kernel_spmd(nc, in_maps, list(range(cfg.NC)))
    o_re = np.concatenate([res.results[r]["o_re"] for r in range(cfg.NC)], axis=0)
    o_im = np.concatenate([res.results[r]["o_im"] for r in range(cfg.NC)], axis=0)
    return (o_re + 1j * o_im).astype(np.complex64)



# revision 30
# speedup vs baseline: 41.0797x; 41.0797x over previous
"""Chopfield attention (complex QKV projections + real-part softmax attention)
on 8 Trainium2 NeuronCores.

Math (reference):
    Q = R @ W_Q ; K = Y @ W_K ; V = Y @ W_V          (complex, [4096,1024])
    Z = BETA * Re(conj(Q) @ K^T)                      [4096,4096] real
      = (BETA*Q_re) @ K_re^T + (BETA*Q_im) @ K_im^T
    A = softmax(Z, axis=-1)                           real
    out = A @ V                                       (complex)

Sharding: queries (R rows) and keys (Y rows) are both sharded 8-way.
Each core computes its K/V shard, AllGathers K^T and V, computes local
Q, scores, softmax and A@V for its 512 query rows.

Precision: the softmax is near-one-hot (score std ~2900), so the whole
Q/K score chain must be fp32-accurate. Score-chain matmuls run in true
fp32 on the PE (measured 2 cycles/row at 512-wide moving operands —
cheaper than the 3-pass fp16 hi/lo split at 3 cycles/row, and exacter).
The V path tolerates fp16 single-pass, as does A@V.
"""

import numpy as np

import concourse.bacc as bacc
import concourse.mybir as mybir
import concourse.tile as tile
from concourse.bass_utils import run_bass_kernel_spmd

BETA = 0.03125
P = 128
FP16 = mybir.dt.float16
FP32 = mybir.dt.float32
X = mybir.AxisListType.X


class Cfg:
    def __init__(self, N=4096, M=4096, D=1024, NC=8):
        self.N, self.M, self.D, self.NC = N, M, D, NC
        self.NL = N // NC          # local query rows
        self.ML = M // NC          # local key rows
        self.DT = D // P           # contraction tiles
        self.QTS = self.NL // P    # local query partition-tiles
        self.MTS = self.ML // P    # local key partition-tiles
        self.DF = min(512, D)      # free-dim chunk for D-wide outputs
        self.DCH = D // self.DF    # chunks of D
        self.MTG = M // P          # global key partition-tiles
        self.KHALF = 2 if self.DT % 2 == 0 else 1   # score K-stream halves
        self.SLOT = D * self.ML    # elements per gathered tensor slot
        # agk slots: 0 = KT_re, 1 = KT_im  ([D, ML] fp32)
        # agv slots: 0 = V_re, 1 = V_im    ([ML, D] fp16)


def build(cfg: Cfg, reps: int = 1, no_collective: bool = False, stop_after: str | None = None):
    c = cfg
    nc = bacc.Bacc("TRN2", target_bir_lowering=False, debug=False, num_devices=c.NC)

    def din(name, shape, dt):
        return nc.dram_tensor(name, shape, dt, kind="ExternalInput")

    # stationary weights (column-sliced per output tile at load time),
    # host-swizzled to [out_block, partition, in_tile*cols] so every
    # per-output-tile slice is one fully-contiguous DMA
    wq = {comp: din(f"wq_{comp}", [c.DT, P, c.DT * P], FP32) for comp in ("re", "im", "s")}
    wk = {comp: din(f"wk_{comp}", [c.DT, P, c.DT * P], FP32) for comp in ("re", "im", "s")}
    wv = {comp: din(f"wv_{comp}", [c.DCH, P, c.DT * c.DF], FP16) for comp in ("re", "im", "s")}

    # moving operands: R^T and Y^T in fp32 (+re+im sum variants for the
    # Karatsuba complex-product decomposition); fp16 Y^T feeds the V proj
    rt = {comp: din(f"rt_{comp}", [P, c.DT * c.NL], FP32) for comp in ("re", "im", "s")}
    yt = {comp: din(f"yt_{comp}", [P, c.DT * c.ML], FP32) for comp in ("re", "im", "s")}
    yt16 = {comp: din(f"yt16_{comp}", [P, c.DT * c.ML], FP16) for comp in ("re", "im", "s")}

    ident = din("ident", [P, P], FP16)

    o_re = nc.dram_tensor("o_re", [c.NL, c.D], FP32, kind="ExternalOutput")
    o_im = nc.dram_tensor("o_im", [c.NL, c.D], FP32, kind="ExternalOutput")

    with tile.TileContext(nc) as tc:
        with (
            tc.tile_pool(name="pers", bufs=1) as pers,
            tc.tile_pool(name="ps", bufs=1, space="PSUM") as ps,
            tc.tile_pool(name="dram", bufs=1, space="DRAM") as dram,
        ):
            def emit(rep):
                prp = tc.alloc_tile_pool(name=f"prp{rep}", bufs=1)
                kvp = tc.alloc_tile_pool(name=f"kvp{rep}", bufs=1)
                _scope_stack = []

                def scope(name):
                    if _scope_stack:
                        pn, pid = _scope_stack.pop()
                        nc.leave_named_scope(pn, pid, False)
                    if name is not None:
                        sid, _ = nc.enter_named_scope(name, False)
                        _scope_stack.append((name, sid))

                # ---------- load Y^T fp32 (moving operand for K projection)
                scope("yload")
                yts = {}
                for comp, t in yt.items():
                    yts[comp] = kvp.tile([P, c.DT * c.ML], FP32, tag=f"yt{comp}", name=f"yt_{comp}_{rep}")
                    nc.scalar.dma_start(yts[comp][:], t.ap())

                ident_sb = pers.tile([P, P], FP16, tag="ident")
                nc.sync.dma_start(ident_sb[:], ident.ap())

                # collective bounce buffers (flat element counts)
                agk_in = dram.tile([2 * c.SLOT], FP32)
                agk_out = dram.tile([c.NC * 2 * c.SLOT], FP32, addr_space="Shared")
                agv_in = dram.tile([2 * c.SLOT], FP16)
                agv_out = dram.tile([c.NC * 2 * c.SLOT], FP16, addr_space="Shared")

                def proj_qk(w, mov, wtag, mw, pool, make_out, post=None):
                    """Karatsuba complex projection, single-pass fp32 products.
                    m1 = A_re@W_re, m2 = A_im@W_im, m3 = (A_re+A_im)@(W_re+W_im);
                    out_re = m1 - m2, out_im = m3 - m1 - m2.
                    make_out(comp, dt_out) -> destination AP [P, mw];
                    post(comp, dt_out, ap) runs after the output is written."""
                    widx = {"re": 0, "im": 1, "s": 2}
                    for dt_out in range(c.DT):
                        wsl = pool.tile([P, 3 * c.DT * P], FP32, tag=wtag, bufs=2)
                        for wc, wi in widx.items():
                            nc.sync.dma_start(
                                wsl[:, wi * c.DT * P : (wi + 1) * c.DT * P],
                                w[wc].ap()[dt_out],
                            )

                        def wslice(wc, ki):
                            wi = widx[wc]
                            return wsl[:, wi * c.DT * P + ki * P : wi * c.DT * P + (ki + 1) * P]

                        m = {}
                        for prod, wc in enumerate(("re", "im", "s")):
                            pt = ps.tile([P, 512], FP32, tag="ps", bufs=6)
                            m[prod] = pt[:, :mw]
                            for ki in range(c.DT):
                                nc.tensor.matmul(
                                    m[prod],
                                    wslice(wc, ki),
                                    mov[wc][:, ki * mw : ki * mw + mw],
                                    start=(ki == 0),
                                    stop=(ki == c.DT - 1),
                                )
                        # DVE may read only ONE operand from PSUM per inst:
                        # stage m2 in SBUF, then chain single-PSUM subtracts.
                        m2s = pool.tile([P, 512], FP32, tag=wtag + "m2s", bufs=2)
                        nc.vector.tensor_copy(m2s[:, :mw], m[1])
                        dre_ap = make_out("re", dt_out)
                        nc.vector.tensor_sub(dre_ap, m[0], m2s[:, :mw])
                        dim = pool.tile([P, 512], FP32, tag=wtag + "dim", bufs=2)
                        nc.vector.tensor_sub(dim[:, :mw], m[2], m2s[:, :mw])
                        dim_ap = make_out("im", dt_out)
                        nc.vector.tensor_sub(dim_ap, dim[:, :mw], m[0])
                        if post is not None:
                            post("re", dt_out, dre_ap)
                            post("im", dt_out, dim_ap)

                # ---------- K^T projection (bounced to DRAM for the AllGather)
                scope("kproj")
                kp = tc.alloc_tile_pool(name=f"kp{rep}", bufs=1)

                def k_make_out(comp, dt_out):
                    t = kp.tile([P, c.ML], FP32, tag=f"kout{comp}", bufs=2)
                    return t[:]

                def k_post(comp, dt_out, ap):
                    si = 0 if comp == "re" else 1
                    dst = agk_in[
                        si * c.SLOT + dt_out * P * c.ML : si * c.SLOT + (dt_out + 1) * P * c.ML
                    ].rearrange("(p m) -> p m", p=P)
                    nc.gpsimd.dma_start(dst, ap)

                proj_qk(wk, yts, "wksl", c.ML, kp, k_make_out, k_post)
                kp.release()
                kvp.release()

                scope("agk")
                if not no_collective:
                    nc.gpsimd.collective_compute(
                        "AllGather",
                        mybir.AluOpType.bypass,
                        replica_groups=[list(range(c.NC))],
                        ins=[agk_in.opt()],
                        outs=[agk_out.opt()],
                    )

                # fp16 Y^T + R^T loads hoisted here: the scalar DMA ring is
                # idle during the projections, so operands land early.
                scope("rload")
                qrt = tc.alloc_tile_pool(name=f"qrt{rep}", bufs=1)
                vp = tc.alloc_tile_pool(name=f"vp{rep}", bufs=1)
                y16 = {}
                for comp, t in yt16.items():
                    y16[comp] = vp.tile([P, c.DT * c.ML], FP16, tag=f"y16{comp}", name=f"y16_{comp}_{rep}")
                    nc.scalar.dma_start(y16[comp][:], t.ap())
                rts = {}
                for comp, t in rt.items():
                    rts[comp] = qrt.tile([P, c.DT * c.NL], FP32, tag=f"rt{comp}", name=f"rt_{comp}_{rep}")
                    nc.scalar.dma_start(rts[comp][:], t.ap())

                # ---------- V projection (single-pass fp16, Karatsuba)
                scope("vproj")
                for dch in range(c.DCH):
                    wvsl = vp.tile([P, 3 * c.DT * c.DF], FP16, tag="wvsl", bufs=2)
                    wvidx = {"re": 0, "im": 1, "s": 2}
                    for wn, wi in wvidx.items():
                        nc.sync.dma_start(
                            wvsl[:, wi * c.DT * c.DF : (wi + 1) * c.DT * c.DF],
                            wv[wn].ap()[dch],
                        )
                    for mt in range(c.MTS):
                        m = {}
                        for prod, yc in enumerate(("re", "im", "s")):
                            pt = ps.tile([P, 512], FP32, tag="ps", bufs=6)
                            m[prod] = pt[:, : c.DF]
                            for ki in range(c.DT):
                                nc.tensor.matmul(
                                    m[prod],
                                    y16[yc][:, ki * c.ML + mt * P : ki * c.ML + (mt + 1) * P],
                                    wvsl[:, wvidx[yc] * c.DT * c.DF + ki * c.DF : wvidx[yc] * c.DT * c.DF + (ki + 1) * c.DF],
                                    start=(ki == 0),
                                    stop=(ki == c.DT - 1),
                                )
                        vm2s = vp.tile([P, c.DF], FP32, tag="vm2s", bufs=2)
                        nc.vector.tensor_copy(vm2s[:], m[1])
                        for comp, si in (("re", 0), ("im", 1)):
                            vout = vp.tile([P, c.DF], FP16, tag="vout", bufs=4)
                            if comp == "re":
                                nc.vector.tensor_sub(vout[:], m[0], vm2s[:])
                            else:
                                vim1 = vp.tile([P, c.DF], FP32, tag="vim1", bufs=2)
                                nc.vector.tensor_sub(vim1[:], m[2], vm2s[:])
                                nc.vector.tensor_sub(vout[:], vim1[:], m[0])
                            dst = agv_in[si * c.SLOT : (si + 1) * c.SLOT].rearrange(
                                "(m p dc d) -> m p dc d", m=c.MTS, p=P, dc=c.DCH
                            )[mt, :, dch, :]
                            nc.gpsimd.dma_start(dst, vout[:])

                # ---------- AllGather V (A@V consumes it much later)
                scope("agv")
                if not no_collective:
                    nc.gpsimd.collective_compute(
                        "AllGather",
                        mybir.AluOpType.bypass,
                        replica_groups=[list(range(c.NC))],
                        ins=[agv_in.opt()],
                        outs=[agv_out.opt()],
                    )
                if stop_after == "vproj":
                    vp.release()
                    qrt.release()
                    prp.release()
                    scope(None)
                    return

                vp.release()

                # ---------- Q^T projection (R^T loads hoisted above)
                scope("qproj")
                qp = tc.alloc_tile_pool(name=f"qp{rep}", bufs=1)
                qt_sb = {}
                for comp in ("re", "im"):
                    qt_sb[comp] = prp.tile([P, c.DT * c.NL], FP32, tag=f"qt{comp}", name=f"qt_{comp}_{rep}")

                def q_make_out(comp, dt_out):
                    return qt_sb[comp][:, dt_out * c.NL : (dt_out + 1) * c.NL]

                proj_qk(wq, rts, "wqsl", c.NL, qp, q_make_out)
                qp.release()
                qrt.release()
                if stop_after == "qproj":
                    prp.release()
                    scope(None)
                    return

                # ---------- scores + streaming softmax (per key-shard chunk)
                # Z[q, m] = QT^T @ KT ; chunk max -> exp(Z - cmax); rescale later.
                # The local shard (r == my rank) reads K^T straight from SBUF —
                # no dependency on the AllGather. Remote shards stream from
                # agk_out.
                scope("scores")
                scp = tc.alloc_tile_pool(name=f"scp{rep}", bufs=1)
                p_sb = [prp.tile([P, c.M], FP16, tag=f"p{qt}", name=f"p_{qt}_{rep}") for qt in range(c.QTS)]
                cm = [prp.tile([P, c.NC], FP32, tag=f"cm{qt}", name=f"cm_{qt}_{rep}") for qt in range(c.QTS)]
                ncm = [prp.tile([P, c.NC], FP32, tag=f"ncm{qt}", name=f"ncm_{qt}_{rep}") for qt in range(c.QTS)]

                def emit_score_chunk(r, ktslice):
                    """ktslice(comp, ki) -> [P, ML] fp32 AP for this shard."""
                    for qt in range(c.QTS):
                        zp = ps.tile([P, 512], FP32, tag="ps", bufs=6)
                        zacc = zp[:, : c.ML]
                        nmm = 2 * c.DT
                        i = 0
                        for comp in ("re", "im"):
                            for ki in range(c.DT):
                                nc.tensor.matmul(
                                    zacc,
                                    qt_sb[comp][:, ki * c.NL + qt * P : ki * c.NL + (qt + 1) * P],
                                    ktslice(comp, ki),
                                    start=(i == 0),
                                    stop=(i == nmm - 1),
                                )
                                i += 1
                        nc.vector.reduce_max(cm[qt][:, r : r + 1], zacc, axis=X)
                        nc.vector.tensor_scalar_mul(
                            ncm[qt][:, r : r + 1], cm[qt][:, r : r + 1], -1.0
                        )
                        nc.scalar.activation(
                            p_sb[qt][:, r * c.ML : (r + 1) * c.ML],
                            zacc,
                            mybir.ActivationFunctionType.Exp,
                            bias=ncm[qt][:, r : r + 1],
                            scale=1.0,
                        )

                kdh = c.DT // c.KHALF  # d-tiles per streamed half
                for r in range(c.NC):
                    halves = []
                    for h in range(c.KHALF):
                        ktl = scp.tile([P, 2 * kdh * c.ML], FP32, tag="ktl", bufs=3)
                        for si in range(2):
                            src = agk_out[
                                r * 2 * c.SLOT
                                + si * c.SLOT
                                + h * kdh * P * c.ML : r * 2 * c.SLOT
                                + si * c.SLOT
                                + (h + 1) * kdh * P * c.ML
                            ].rearrange("(t p m) -> p t m", p=P, m=c.ML)
                            nc.scalar.dma_start(
                                ktl[:, si * kdh * c.ML : (si + 1) * kdh * c.ML].rearrange(
                                    "p (t m) -> p t m", m=c.ML
                                ),
                                src,
                            )
                        halves.append(ktl)

                    def ktslice(comp, ki, halves=halves):
                        si = {"re": 0, "im": 1}[comp]
                        t = halves[ki // kdh]
                        k = ki % kdh
                        return t[:, si * kdh * c.ML + k * c.ML : si * kdh * c.ML + (k + 1) * c.ML]

                    emit_score_chunk(r, ktslice)

                # ---------- finalize softmax: rescale chunks to the global max
                scope("smax")
                recip = []
                for qt in range(c.QTS):
                    ngm = prp.tile([P, 1], FP32, tag=f"ngm{qt}")
                    nc.vector.tensor_reduce(
                        ngm[:], ncm[qt][:], op=mybir.AluOpType.min, axis=X
                    )
                    fac = prp.tile([P, c.NC], FP32, tag=f"fac{qt}")
                    nc.scalar.activation(
                        fac[:],
                        ncm[qt][:],
                        mybir.ActivationFunctionType.Exp,
                        bias=ngm[:, 0:1],
                        scale=-1.0,
                    )
                    for r in range(c.NC):
                        nc.vector.tensor_scalar_mul(
                            p_sb[qt][:, r * c.ML : (r + 1) * c.ML],
                            p_sb[qt][:, r * c.ML : (r + 1) * c.ML],
                            fac[:, r : r + 1],
                        )
                    ssum = prp.tile([P, 1], FP32, tag=f"ssum{qt}")
                    nc.vector.reduce_sum(ssum[:], p_sb[qt][:], axis=X)
                    rc = prp.tile([P, 1], FP32, tag=f"rcp{qt}")
                    nc.vector.reciprocal(rc[:], ssum[:])
                    recip.append(rc)

                scp.release()
                if stop_after == "scores":
                    prp.release()
                    scope(None)
                    return

                # ---------- transpose P -> P^T tiles ([m-part, q-free])
                scope("transp")
                avp = tc.alloc_tile_pool(name=f"avp{rep}", bufs=1)
                pt_sb = [avp.tile([P, c.NL], FP16, tag=f"pt{mtg}", name=f"pt_{mtg}_{rep}") for mtg in range(c.MTG)]
                for mtg in range(c.MTG):
                    tp = ps.tile([P, 512], FP16, tag="dsc", bufs=2)
                    tacc = tp[:, : c.NL]
                    for qt in range(c.QTS):
                        nc.tensor.matmul(
                            tacc[:, qt * P : (qt + 1) * P],
                            p_sb[qt][:, mtg * P : (mtg + 1) * P],
                            ident_sb[:],
                            start=True,
                            stop=True,
                            is_transpose=True,
                        )
                    nc.vector.tensor_copy(pt_sb[mtg][:], tacc)
                if stop_after == "transp":
                    avp.release()
                    prp.release()
                    scope(None)
                    return

                # ---------- A @ V (+ 1/sum scaling)
                scope("av")
                for comp, odram in (("re", o_re), ("im", o_im)):
                    si = 0 if comp == "re" else 1
                    for dch in range(c.DCH):
                        vh = avp.tile([P, c.MTG * c.DF], FP16, tag="vh", bufs=2)
                        for r in range(c.NC):
                            src = agv_out[
                                r * 2 * c.SLOT + si * c.SLOT : r * 2 * c.SLOT + (si + 1) * c.SLOT
                            ].rearrange("(m p dc d) -> dc p m d", m=c.MTS, p=P, dc=c.DCH)[dch]
                            nc.sync.dma_start(
                                vh[
                                    :, r * c.MTS * c.DF : (r + 1) * c.MTS * c.DF
                                ].rearrange("p (m d) -> p m d", m=c.MTS),
                                src,
                            )
                        for qt in range(c.QTS):
                            op_ = ps.tile([P, 512], FP32, tag="ps", bufs=6)
                            oacc = op_[:, : c.DF]
                            for mtg in range(c.MTG):
                                nc.tensor.matmul(
                                    oacc,
                                    pt_sb[mtg][:, qt * P : (qt + 1) * P],
                                    vh[:, mtg * c.DF : (mtg + 1) * c.DF],
                                    start=(mtg == 0),
                                    stop=(mtg == c.MTG - 1),
                                )
                            osb = avp.tile([P, c.DF], FP32, tag="osb", bufs=4)
                            nc.vector.tensor_scalar_mul(osb[:], oacc, recip[qt][:, 0:1])
                            nc.sync.dma_start(
                                odram.ap()[
                                    qt * P : (qt + 1) * P, dch * c.DF : (dch + 1) * c.DF
                                ],
                                osb[:],
                            )
                avp.release()
                prp.release()
                scope(None)

            for rep in range(reps):
                emit(rep)

    nc.compile()
    return nc


def prep_inputs(cfg, R_re, R_im, Y_re, Y_im, W_Q_re, W_Q_im, W_K_re, W_K_im, W_V_re, W_V_im):
    """Host-side sharding + transposes + weight swizzles. Returns in_maps."""
    c = cfg
    f32 = np.float32
    wq_re = np.ascontiguousarray(W_Q_re, dtype=f32) * BETA
    wq_im = np.ascontiguousarray(W_Q_im, dtype=f32) * BETA
    wk_re = np.ascontiguousarray(W_K_re, dtype=f32)
    wk_im = np.ascontiguousarray(W_K_im, dtype=f32)
    wv_re = np.ascontiguousarray(W_V_re, dtype=f32)
    wv_im = np.ascontiguousarray(W_V_im, dtype=f32)
    ident = np.eye(P, dtype=np.float16)

    DT, DCH, DF = cfg.DT, cfg.DCH, cfg.DF

    def _wsw(w, ocols):
        # [d_in, d_out] -> [d_out_block, p, d_in_tile * ocols], contiguous
        ob = w.shape[1] // ocols
        return np.ascontiguousarray(
            w.reshape(DT, P, ob, ocols).transpose(2, 1, 0, 3).reshape(ob, P, DT * ocols)
        )

    shared = {}
    for comp, arr in (("re", wq_re), ("im", wq_im), ("s", wq_re + wq_im)):
        shared[f"wq_{comp}"] = _wsw(arr, P)
    for comp, arr in (("re", wk_re), ("im", wk_im), ("s", wk_re + wk_im)):
        shared[f"wk_{comp}"] = _wsw(arr, P)
    shared["wv_re"] = _wsw(wv_re.astype(np.float16), DF)
    shared["wv_im"] = _wsw(wv_im.astype(np.float16), DF)
    shared["wv_s"] = _wsw((wv_re + wv_im).astype(np.float16), DF)
    shared["ident"] = ident

    in_maps = []
    for r in range(c.NC):
        m = dict(shared)
        rsl = slice(r * c.NL, (r + 1) * c.NL)
        ysl = slice(r * c.ML, (r + 1) * c.ML)
        rre_t = np.ascontiguousarray(np.asarray(R_re[rsl], dtype=f32).T)
        rim_t = np.ascontiguousarray(np.asarray(R_im[rsl], dtype=f32).T)
        yre_t = np.ascontiguousarray(np.asarray(Y_re[ysl], dtype=f32).T)
        yim_t = np.ascontiguousarray(np.asarray(Y_im[ysl], dtype=f32).T)
        for base, arr in (("rt_re", rre_t), ("rt_im", rim_t), ("rt_s", rre_t + rim_t),
                          ("yt_re", yre_t), ("yt_im", yim_t), ("yt_s", yre_t + yim_t)):
            mw = arr.shape[1]
            sw = np.ascontiguousarray(
                arr.reshape(DT, P, mw).transpose(1, 0, 2).reshape(P, DT * mw)
            )
            m[base] = sw
            if base.startswith("yt"):
                m["yt16" + base[2:]] = sw.astype(np.float16)
        in_maps.append(m)
    return in_maps


_NC_CACHE = {}


def kernel(**inputs) -> np.ndarray:
    cfg = Cfg()
    if "full" not in _NC_CACHE:
        _NC_CACHE["full"] = build(cfg, 1)
    nc = _NC_CACHE["full"]
    in_maps = prep_inputs(cfg, **inputs)
    res = run_bass_kernel_spmd(nc, in_maps, list(range(cfg.NC)))
    o_re = np.concatenate([res.results[r]["o_re"] for r in range(cfg.NC)], axis=0)
    o_im = np.concatenate([res.results[r]["o_im"] for r in range(cfg.NC)], axis=0)
    return (o_re + 1j * o_im).astype(np.complex64)


# revision 32
# speedup vs baseline: 51.0112x; 1.2418x over previous
"""Chopfield attention (complex QKV projections + real-part softmax attention)
on 8 Trainium2 NeuronCores.

Math (reference):
    Q = R @ W_Q ; K = Y @ W_K ; V = Y @ W_V          (complex, [4096,1024])
    Z = BETA * Re(conj(Q) @ K^T)                      [4096,4096] real
      = (BETA*Q_re) @ K_re^T + (BETA*Q_im) @ K_im^T
    A = softmax(Z, axis=-1)                           real
    out = A @ V                                       (complex)

Sharding: queries (R rows) and keys (Y rows) are both sharded 8-way.
Each core computes its K/V shard, AllGathers K^T and V, computes local
Q, scores, softmax and A@V for its 512 query rows.

Precision: the softmax is near-one-hot (score std ~2900), so the whole
Q/K score chain must be fp32-accurate. All score-chain matmuls use a
3-pass fp16 hi/lo split (fp16 products are exact on the PE and
accumulate in fp32), which lands within ~2e-3 of a pure-fp32 pipeline.
The V path tolerates fp16 single-pass.
"""

import numpy as np

import concourse.bacc as bacc
import concourse.mybir as mybir
import concourse.tile as tile
from concourse.bass_utils import run_bass_kernel_spmd

BETA = 0.03125
P = 128
FP16 = mybir.dt.float16
FP32 = mybir.dt.float32
X = mybir.AxisListType.X


class Cfg:
    def __init__(self, N=4096, M=4096, D=1024, NC=8):
        self.N, self.M, self.D, self.NC = N, M, D, NC
        self.NL = N // NC          # local query rows
        self.ML = M // NC          # local key rows
        self.DT = D // P           # contraction tiles
        self.QTS = self.NL // P    # local query partition-tiles
        self.MTS = self.ML // P    # local key partition-tiles
        self.DF = min(512, D)      # free-dim chunk for D-wide outputs
        self.DCH = D // self.DF    # chunks of D
        self.MTG = M // P          # global key partition-tiles
        self.KHALF = 2 if self.DT % 2 == 0 else 1   # score K-stream halves
        self.SLOT = D * self.ML    # elements per gathered tensor slot
        # slots: 0..3 = KT(re_h, re_l, im_h, im_l) [D, ML]; 4,5 = V(re, im) [ML, D]
        self.NSLOT = 6


def build(cfg: Cfg, reps: int = 1, no_collective: bool = False, stop_after: str | None = None):
    c = cfg
    nc = bacc.Bacc("TRN2", target_bir_lowering=False, debug=False, num_devices=c.NC)

    def din(name, shape, dt=FP16):
        return nc.dram_tensor(name, shape, dt, kind="ExternalInput")

    # stationary weights [D, D] (column-sliced per output tile at load time)
    # weights host-swizzled to [out_block, partition, in_tile*cols] so every
    # per-output-tile slice is one fully-contiguous DMA
    wq = {}
    for comp in ("re", "im", "s"):
        for lvl in ("h", "l"):
            wq[comp, lvl] = din(f"wq_{comp}_{lvl}", [c.DT, P, c.DT * P])
    wk = {}
    for comp in ("re", "im", "s"):
        for lvl in ("h", "l"):
            wk[comp, lvl] = din(f"wk_{comp}_{lvl}", [c.DT, P, c.DT * P])
    wv = {n: din(f"wv_{n}", [c.DCH, P, c.DT * c.DF]) for n in ("re", "im", "s")}

    # moving operands: R^T and Y^T with hi/lo splits (+re+im sum variants
    # for the Karatsuba complex-product decomposition)
    rt = {}
    yt = {}
    for comp in ("re", "im", "s"):
        for lvl in ("h", "l"):
            rt[comp, lvl] = din(f"rt_{comp}_{lvl}", [P, c.DT * c.NL])
            yt[comp, lvl] = din(f"yt_{comp}_{lvl}", [P, c.DT * c.ML])

    ident = din("ident", [P, P])

    o_re = nc.dram_tensor("o_re", [c.NL, c.D], FP32, kind="ExternalOutput")
    o_im = nc.dram_tensor("o_im", [c.NL, c.D], FP32, kind="ExternalOutput")

    with tile.TileContext(nc) as tc:
        with (
            tc.tile_pool(name="pers", bufs=1) as pers,
            tc.tile_pool(name="ps", bufs=1, space="PSUM") as ps,
            tc.tile_pool(name="dram", bufs=1, space="DRAM") as dram,
        ):
            def emit(rep):
                prp = tc.alloc_tile_pool(name=f"prp{rep}", bufs=1)
                kvp = tc.alloc_tile_pool(name=f"kvp{rep}", bufs=1)
                qrt = tc.alloc_tile_pool(name=f"qrt{rep}", bufs=1)
                # ---------- load Y^T (moving operand for K and V projections)
                _scope_stack = []

                def scope(name):
                    if _scope_stack:
                        pn, pid = _scope_stack.pop()
                        nc.leave_named_scope(pn, pid, False)
                    if name is not None:
                        sid, _ = nc.enter_named_scope(name, False)
                        _scope_stack.append((name, sid))

                scope("yload")
                yts = {}
                for key, t in yt.items():
                    yts[key] = kvp.tile([P, c.DT * c.ML], FP16, tag=f"yt{key}", name=f"yt_{key[0]}_{key[1]}_{rep}")
                    nc.scalar.dma_start(yts[key][:], t.ap())

                ident_sb = pers.tile([P, P], FP16, tag="ident")
                nc.sync.dma_start(ident_sb[:], ident.ap())

                # R^T loads hoisted here: they overlap the K projection, so
                # Q's moving operands are resident before qproj starts.
                scope("rload")
                rts = {}
                for key, t in rt.items():
                    rts[key] = qrt.tile([P, c.DT * c.NL], FP16, tag=f"rt{key}", name=f"rt_{key[0]}_{key[1]}_{rep}")
                    nc.scalar.dma_start(rts[key][:], t.ap())

                # collective bounce buffers (flat fp16 element counts)
                agk_in = dram.tile([4 * c.SLOT], FP16)
                agk_out = dram.tile([c.NC * 4 * c.SLOT], FP16, addr_space="Shared")
                agv_in = dram.tile([2 * c.SLOT], FP16)
                agv_out = dram.tile([c.NC * 2 * c.SLOT], FP16, addr_space="Shared")

                # ---------- K^T projection: K^T = W_K^T @ Y^T  (3-pass split)
                # K_re^T = WKre^T@YTre + WKim^T@YTimn ; K_im^T = WKim^T@YTre + WKre^T@YTim

                def proj_qk(w, mov, wtag, mw, pool, out_sb=None, bounce_si=None):
                    """Karatsuba complex projection, 3-pass fp16 split per product.
                    m1 = A_re@W_re, m2 = A_im@W_im, m3 = (A_re+A_im)@(W_re+W_im);
                    out_re = m1 - m2, out_im = m3 - m1 - m2."""
                    for dt_out in range(c.DT):
                        wsl = pool.tile([P, 6 * c.DT * P], FP16, tag=wtag, bufs=2)
                        widx = {("re", "h"): 0, ("re", "l"): 1, ("im", "h"): 2,
                                ("im", "l"): 3, ("s", "h"): 4, ("s", "l"): 5}
                        for (wc, wl), wi in widx.items():
                            nc.sync.dma_start(
                                wsl[:, wi * c.DT * P : (wi + 1) * c.DT * P],
                                w[wc, wl].ap()[dt_out],
                            )

                        def wslice(wc, wl, ki):
                            wi = widx[wc, wl]
                            return wsl[:, wi * c.DT * P + ki * P : wi * c.DT * P + (ki + 1) * P]

                        m = {}
                        for prod, (wc, mc) in enumerate(
                            [("re", "re"), ("im", "im"), ("s", "s")]
                        ):
                            pt = ps.tile([P, 512], FP32, tag="ps", bufs=6)
                            m[prod] = pt[:, :mw]
                            nmm = c.DT * 3
                            i = 0
                            for ki in range(c.DT):
                                for wl, ml in (("h", "h"), ("h", "l"), ("l", "h")):
                                    nc.tensor.matmul(
                                        m[prod],
                                        wslice(wc, wl, ki),
                                        mov[mc, ml][:, ki * mw : ki * mw + mw],
                                        start=(i == 0),
                                        stop=(i == nmm - 1),
                                    )
                                    i += 1
                        # DVE may read only ONE operand from PSUM per inst:
                        # stage m2 in SBUF, then chain single-PSUM subtracts.
                        m2s = pool.tile([P, 512], FP32, tag=wtag + "m2s", bufs=2)
                        nc.vector.tensor_copy(m2s[:, :mw], m[1])
                        dre = pool.tile([P, 512], FP32, tag=wtag + "dre", bufs=2)
                        nc.vector.tensor_sub(dre[:, :mw], m[0], m2s[:, :mw])
                        dim = pool.tile([P, 512], FP32, tag=wtag + "dim", bufs=2)
                        nc.vector.tensor_sub(dim[:, :mw], m[2], m2s[:, :mw])
                        nc.vector.tensor_sub(dim[:, :mw], dim[:, :mw], m[0])
                        for comp, d in (("re", dre), ("im", dim)):
                            if out_sb is not None:
                                hi = out_sb[comp, "h"][:, dt_out * mw : (dt_out + 1) * mw]
                                lo = out_sb[comp, "l"][:, dt_out * mw : (dt_out + 1) * mw]
                            else:
                                hi = pool.tile([P, mw], FP16, tag=wtag + comp + "hi", bufs=2)
                                lo = pool.tile([P, mw], FP16, tag=wtag + comp + "lo", bufs=2)
                            nc.vector.tensor_copy(hi if out_sb is not None else hi[:], d[:, :mw])
                            nc.vector.tensor_sub(lo if out_sb is not None else lo[:], d[:, :mw], hi if out_sb is not None else hi[:])
                            if out_sb is None:
                                for lvl, t in (("h", hi), ("l", lo)):
                                    si = bounce_si[comp, lvl]
                                    dst = agk_in[
                                        si * c.SLOT + dt_out * P * mw : si * c.SLOT + (dt_out + 1) * P * mw
                                    ].rearrange("(p m) -> p m", p=P)
                                    nc.gpsimd.dma_start(dst, t[:])

                kp = tc.alloc_tile_pool(name=f"kp{rep}", bufs=1)
                scope("kproj")
                proj_qk(
                    wk, yts, "wksl", c.ML, kp,
                    bounce_si={("re", "h"): 0, ("re", "l"): 1, ("im", "h"): 2, ("im", "l"): 3},
                )
                kp.release()
                scope("agk")
                if not no_collective:
                    nc.gpsimd.collective_compute(
                        "AllGather",
                        mybir.AluOpType.bypass,
                        replica_groups=[list(range(c.NC))],
                        ins=[agk_in.opt()],
                        outs=[agk_out.opt()],
                    )

                # ---------- Q^T projection (R^T loads hoisted above, before V proj)
                scope("qproj")
                qp = tc.alloc_tile_pool(name=f"qp{rep}", bufs=1)
                qt_sb = {}
                for comp in ("re", "im"):
                    for lvl in ("h", "l"):
                        qt_sb[comp, lvl] = prp.tile([P, c.DT * c.NL], FP16, tag=f"qt{comp}{lvl}", name=f"qt_{comp}_{lvl}_{rep}")
                proj_qk(wq, rts, "wqsl", c.NL, qp, out_sb=qt_sb)
                qp.release()
                qrt.release()
                if stop_after == "qproj":
                    prp.release()
                    scope(None)
                    return

                vp = tc.alloc_tile_pool(name=f"vp{rep}", bufs=1)
                scope("vproj")
                # ---------- V projection (single-pass fp16, Karatsuba):
                # m1 = Yre@WVre, m2 = Yim@WVim, m3 = (Yre+Yim)@(WVre+WVim)
                # V_re = m1 - m2 ; V_im = m3 - m1 - m2
                for dch in range(c.DCH):
                    wvsl = vp.tile([P, 3 * c.DT * c.DF], FP16, tag="wvsl", bufs=2)
                    wvidx = {"re": 0, "im": 1, "s": 2}
                    for wn, wi in wvidx.items():
                        nc.sync.dma_start(
                            wvsl[:, wi * c.DT * c.DF : (wi + 1) * c.DT * c.DF],
                            wv[wn].ap()[dch],
                        )
                    for mt in range(c.MTS):
                        m = {}
                        for prod, yc in enumerate(("re", "im", "s")):
                            pt = ps.tile([P, 512], FP32, tag="ps", bufs=6)
                            m[prod] = pt[:, : c.DF]
                            wn = yc
                            for ki in range(c.DT):
                                nc.tensor.matmul(
                                    m[prod],
                                    yts[yc, "h"][:, ki * c.ML + mt * P : ki * c.ML + (mt + 1) * P],
                                    wvsl[:, wvidx[wn] * c.DT * c.DF + ki * c.DF : wvidx[wn] * c.DT * c.DF + (ki + 1) * c.DF],
                                    start=(ki == 0),
                                    stop=(ki == c.DT - 1),
                                )
                        vm2s = vp.tile([P, c.DF], FP32, tag="vm2s", bufs=2)
                        nc.vector.tensor_copy(vm2s[:], m[1])
                        for comp, si in (("re", 0), ("im", 1)):
                            vout = vp.tile([P, c.DF], FP16, tag="vout", bufs=4)
                            if comp == "re":
                                nc.vector.tensor_sub(vout[:], m[0], vm2s[:])
                            else:
                                vim1 = vp.tile([P, c.DF], FP32, tag="vim1", bufs=2)
                                nc.vector.tensor_sub(vim1[:], m[2], vm2s[:])
                                nc.vector.tensor_sub(vout[:], vim1[:], m[0])
                            dst = agv_in[si * c.SLOT : (si + 1) * c.SLOT].rearrange(
                                "(m p dc d) -> m p dc d", m=c.MTS, p=P, dc=c.DCH
                            )[mt, :, dch, :]
                            nc.gpsimd.dma_start(dst, vout[:])

                # ---------- AllGather V (A@V consumes it much later)
                scope("agv")
                if not no_collective:
                    nc.gpsimd.collective_compute(
                        "AllGather",
                        mybir.AluOpType.bypass,
                        replica_groups=[list(range(c.NC))],
                        ins=[agv_in.opt()],
                        outs=[agv_out.opt()],
                    )
                if stop_after == "vproj":
                    vp.release()
                    kvp.release()
                    prp.release()
                    scope(None)
                    return

                vp.release()
                kvp.release()


                # ---------- scores + streaming softmax (per key-shard chunk)
                # Z[q, m] = QT^T @ KT ; chunk max -> exp(Z - cmax); rescale later.
                scope("scores")
                scp = tc.alloc_tile_pool(name=f"scp{rep}", bufs=1)
                p_sb = [prp.tile([P, c.M], FP16, tag=f"p{qt}", name=f"p_{qt}_{rep}") for qt in range(c.QTS)]
                cm = [prp.tile([P, c.NC], FP32, tag=f"cm{qt}", name=f"cm_{qt}_{rep}") for qt in range(c.QTS)]
                ncm = [prp.tile([P, c.NC], FP32, tag=f"ncm{qt}", name=f"ncm_{qt}_{rep}") for qt in range(c.QTS)]

                kdh = c.DT // c.KHALF  # d-tiles per streamed half
                for r in range(c.NC):
                    halves = []
                    for h in range(c.KHALF):
                        ktl = scp.tile([P, 4 * kdh * c.ML], FP16, tag="ktl", bufs=3)
                        for si in range(4):
                            src = agk_out[
                                r * 4 * c.SLOT
                                + si * c.SLOT
                                + h * kdh * P * c.ML : r * 4 * c.SLOT
                                + si * c.SLOT
                                + (h + 1) * kdh * P * c.ML
                            ].rearrange("(t p m) -> p t m", p=P, m=c.ML)
                            nc.scalar.dma_start(
                                ktl[:, si * kdh * c.ML : (si + 1) * kdh * c.ML].rearrange(
                                    "p (t m) -> p t m", m=c.ML
                                ),
                                src,
                            )
                        halves.append(ktl)

                    def ktslice(comp, lvl, ki):
                        si = {("re", "h"): 0, ("re", "l"): 1, ("im", "h"): 2, ("im", "l"): 3}[comp, lvl]
                        t = halves[ki // kdh]
                        k = ki % kdh
                        return t[:, si * kdh * c.ML + k * c.ML : si * kdh * c.ML + (k + 1) * c.ML]

                    for qt in range(c.QTS):
                        zp = ps.tile([P, 512], FP32, tag="ps", bufs=6)
                        zacc = zp[:, : c.ML]
                        nmm = 2 * c.DT * 3
                        i = 0
                        for comp in ("re", "im"):
                            for ki in range(c.DT):
                                for ql, kl in (("h", "h"), ("h", "l"), ("l", "h")):
                                    nc.tensor.matmul(
                                        zacc,
                                        qt_sb[comp, ql][:, ki * c.NL + qt * P : ki * c.NL + (qt + 1) * P],
                                        ktslice(comp, kl, ki),
                                        start=(i == 0),
                                        stop=(i == nmm - 1),
                                    )
                                    i += 1
                        nc.vector.reduce_max(cm[qt][:, r : r + 1], zacc, axis=X)
                        nc.vector.tensor_scalar_mul(
                            ncm[qt][:, r : r + 1], cm[qt][:, r : r + 1], -1.0
                        )
                        nc.scalar.activation(
                            p_sb[qt][:, r * c.ML : (r + 1) * c.ML],
                            zacc,
                            mybir.ActivationFunctionType.Exp,
                            bias=ncm[qt][:, r : r + 1],
                            scale=1.0,
                        )

                # ---------- finalize softmax: rescale chunks to the global max
                scope("smax")
                recip = []
                for qt in range(c.QTS):
                    ngm = prp.tile([P, 1], FP32, tag=f"ngm{qt}")
                    nc.vector.tensor_reduce(
                        ngm[:], ncm[qt][:], op=mybir.AluOpType.min, axis=X
                    )
                    fac = prp.tile([P, c.NC], FP32, tag=f"fac{qt}")
                    nc.scalar.activation(
                        fac[:],
                        ncm[qt][:],
                        mybir.ActivationFunctionType.Exp,
                        bias=ngm[:, 0:1],
                        scale=-1.0,
                    )
                    for r in range(c.NC):
                        nc.vector.tensor_scalar_mul(
                            p_sb[qt][:, r * c.ML : (r + 1) * c.ML],
                            p_sb[qt][:, r * c.ML : (r + 1) * c.ML],
                            fac[:, r : r + 1],
                        )
                    ssum = prp.tile([P, 1], FP32, tag=f"ssum{qt}")
                    nc.vector.reduce_sum(ssum[:], p_sb[qt][:], axis=X)
                    rc = prp.tile([P, 1], FP32, tag=f"rcp{qt}")
                    nc.vector.reciprocal(rc[:], ssum[:])
                    recip.append(rc)

                scp.release()
                if stop_after == "scores":
                    prp.release()
                    scope(None)
                    return

                # ---------- transpose P -> P^T tiles ([m-part, q-free])
                scope("transp")
                avp = tc.alloc_tile_pool(name=f"avp{rep}", bufs=1)
                pt_sb = [avp.tile([P, c.NL], FP16, tag=f"pt{mtg}", name=f"pt_{mtg}_{rep}") for mtg in range(c.MTG)]
                for mtg in range(c.MTG):
                    tp = ps.tile([P, 512], FP16, tag="dsc", bufs=2)
                    tacc = tp[:, : c.NL]
                    for qt in range(c.QTS):
                        nc.tensor.matmul(
                            tacc[:, qt * P : (qt + 1) * P],
                            p_sb[qt][:, mtg * P : (mtg + 1) * P],
                            ident_sb[:],
                            start=True,
                            stop=True,
                            is_transpose=True,
                        )
                    nc.vector.tensor_copy(pt_sb[mtg][:], tacc)
                if stop_after == "transp":
                    avp.release()
                    prp.release()
                    scope(None)
                    return

                # ---------- A @ V (+ 1/sum scaling)
                scope("av")
                for comp, odram in (("re", o_re), ("im", o_im)):
                    si = 0 if comp == "re" else 1
                    for dch in range(c.DCH):
                        vh = avp.tile([P, c.MTG * c.DF], FP16, tag="vh", bufs=2)
                        for r in range(c.NC):
                            src = agv_out[
                                r * 2 * c.SLOT + si * c.SLOT : r * 2 * c.SLOT + (si + 1) * c.SLOT
                            ].rearrange("(m p dc d) -> dc p m d", m=c.MTS, p=P, dc=c.DCH)[dch]
                            nc.sync.dma_start(
                                vh[
                                    :, r * c.MTS * c.DF : (r + 1) * c.MTS * c.DF
                                ].rearrange("p (m d) -> p m d", m=c.MTS),
                                src,
                            )
                        for qt in range(c.QTS):
                            op_ = ps.tile([P, 512], FP32, tag="ps", bufs=6)
                            oacc = op_[:, : c.DF]
                            for mtg in range(c.MTG):
                                nc.tensor.matmul(
                                    oacc,
                                    pt_sb[mtg][:, qt * P : (qt + 1) * P],
                                    vh[:, mtg * c.DF : (mtg + 1) * c.DF],
                                    start=(mtg == 0),
                                    stop=(mtg == c.MTG - 1),
                                )
                            osb = avp.tile([P, c.DF], FP32, tag="osb", bufs=4)
                            nc.vector.tensor_scalar_mul(osb[:], oacc, recip[qt][:, 0:1])
                            nc.sync.dma_start(
                                odram.ap()[
                                    qt * P : (qt + 1) * P, dch * c.DF : (dch + 1) * c.DF
                                ],
                                osb[:],
                            )
                avp.release()
                prp.release()
                scope(None)

            for rep in range(reps):
                emit(rep)

    nc.compile()
    return nc


def _split16(x):
    h = x.astype(np.float16)
    l = (x - h.astype(np.float32)).astype(np.float16)
    return h, l


def prep_inputs(cfg, R_re, R_im, Y_re, Y_im, W_Q_re, W_Q_im, W_K_re, W_K_im, W_V_re, W_V_im):
    """Host-side sharding + fp16 hi/lo split + transposes. Returns in_maps."""
    c = cfg
    f32 = np.float32
    wq_re = np.ascontiguousarray(W_Q_re, dtype=f32) * BETA
    wq_im = np.ascontiguousarray(W_Q_im, dtype=f32) * BETA
    wk_re = np.ascontiguousarray(W_K_re, dtype=f32)
    wk_im = np.ascontiguousarray(W_K_im, dtype=f32)
    wv_re = np.ascontiguousarray(W_V_re, dtype=f32)
    wv_im = np.ascontiguousarray(W_V_im, dtype=f32)
    wqs = {"re": _split16(wq_re), "im": _split16(wq_im), "s": _split16(wq_re + wq_im)}
    wks = {"re": _split16(wk_re), "im": _split16(wk_im), "s": _split16(wk_re + wk_im)}
    ident = np.eye(P, dtype=np.float16)

    DT, DCH, DF = cfg.DT, cfg.DCH, cfg.DF

    def _wsw(w16, ocols):
        # [d_in, d_out] -> [d_out_block, p, d_in_tile * ocols], contiguous
        ob = w16.shape[1] // ocols
        return np.ascontiguousarray(
            w16.reshape(DT, P, ob, ocols).transpose(2, 1, 0, 3).reshape(ob, P, DT * ocols)
        )

    shared = {}
    for comp in ("re", "im", "s"):
        for li, lvl in enumerate(("h", "l")):
            shared[f"wq_{comp}_{lvl}"] = _wsw(wqs[comp][li], P)
            shared[f"wk_{comp}_{lvl}"] = _wsw(wks[comp][li], P)
    shared["wv_re"] = _wsw(wv_re.astype(np.float16), DF)
    shared["wv_im"] = _wsw(wv_im.astype(np.float16), DF)
    shared["wv_s"] = _wsw((wv_re + wv_im).astype(np.float16), DF)
    shared["ident"] = ident

    in_maps = []
    for r in range(c.NC):
        m = dict(shared)
        rsl = slice(r * c.NL, (r + 1) * c.NL)
        ysl = slice(r * c.ML, (r + 1) * c.ML)
        rre_t = np.ascontiguousarray(np.asarray(R_re[rsl], dtype=f32).T)
        rim_t = np.ascontiguousarray(np.asarray(R_im[rsl], dtype=f32).T)
        yre_t = np.ascontiguousarray(np.asarray(Y_re[ysl], dtype=f32).T)
        yim_t = np.ascontiguousarray(np.asarray(Y_im[ysl], dtype=f32).T)
        for base, arr in (("rt_re", rre_t), ("rt_im", rim_t), ("rt_s", rre_t + rim_t),
                          ("yt_re", yre_t), ("yt_im", yim_t), ("yt_s", yre_t + yim_t)):
            h, l = _split16(arr)
            mw = arr.shape[1]
            for lvl, a in (("h", h), ("l", l)):
                m[f"{base}_{lvl}"] = np.ascontiguousarray(
                    a.reshape(DT, P, mw).transpose(1, 0, 2).reshape(P, DT * mw)
                )
        in_maps.append(m)
    return in_maps


_NC_CACHE = {}


def kernel(**inputs) -> np.ndarray:
    cfg = Cfg()
    if "full" not in _NC_CACHE:
        _NC_CACHE["full"] = build(cfg, 1)
    nc = _NC_CACHE["full"]
    in_maps = prep_inputs(cfg, **inputs)
    res = run_bass_kernel_spmd(nc, in_maps, list(range(cfg.NC)))
    o_re = np.concatenate([res.results[r]["o_re"] for r in range(cfg.NC)], axis=0)
    o_im = np.concatenate([res.results[r]["o_im"] for r in range(cfg.NC)], axis=0)
    return (o_re + 1j * o_im).astype(np.complex64)



# revision 35
# speedup vs baseline: 53.9997x; 1.0586x over previous
"""Chopfield attention (complex QKV projections + real-part softmax attention)
on 8 Trainium2 NeuronCores.

Math (reference):
    Q = R @ W_Q ; K = Y @ W_K ; V = Y @ W_V          (complex, [4096,1024])
    Z = BETA * Re(conj(Q) @ K^T)                      [4096,4096] real
      = (BETA*Q_re) @ K_re^T + (BETA*Q_im) @ K_im^T
    A = softmax(Z, axis=-1)                           real
    out = A @ V                                       (complex)

Sharding: queries (R rows) and keys (Y rows) are both sharded 8-way.

G-trick: Z = Re(conj(R @ W_Q) @ (Y @ W_K)^T) = Re(conj(R) @ G @ Y^T) with
G = BETA * conj(W_Q) @ W_K^T precomputed on the HOST (weights only). The
device never materializes K: it AllGathers raw Y^T (host pre-split hi/lo,
bounced DRAM->DRAM at rep start, so the collective depends on no compute),
computes T = conj(R) @ G for its local query rows, then scores
Z = T_re @ Y_re^T + T_im' @ Y_im^T with T_im' = -Im(T).
Each core also computes its V shard and AllGathers V for A@V.

Precision: the softmax is near-one-hot (score std ~2900), so the whole
score chain must be fp32-accurate. G is computed in float64 on host and
split hi/lo fp16; all score-chain matmuls use a 3-pass fp16 hi/lo split
(fp16 products are exact on the PE and accumulate in fp32), landing
within ~2e-3 of a pure-fp32 pipeline. The V path tolerates fp16.
"""

import numpy as np

import concourse.bacc as bacc
import concourse.mybir as mybir
import concourse.tile as tile
from concourse.bass_utils import run_bass_kernel_spmd

BETA = 0.03125
P = 128
FP16 = mybir.dt.float16
FP32 = mybir.dt.float32
X = mybir.AxisListType.X


class Cfg:
    def __init__(self, N=4096, M=4096, D=1024, NC=8):
        self.N, self.M, self.D, self.NC = N, M, D, NC
        self.NL = N // NC          # local query rows
        self.ML = M // NC          # local key rows
        self.DT = D // P           # contraction tiles
        self.QTS = self.NL // P    # local query partition-tiles
        self.MTS = self.ML // P    # local key partition-tiles
        self.DF = min(512, D)      # free-dim chunk for D-wide outputs
        self.DCH = D // self.DF    # chunks of D
        self.MTG = M // P          # global key partition-tiles
        self.KHALF = 2 if self.DT % 2 == 0 else 1   # score K-stream halves
        self.SLOT = D * self.ML    # elements per gathered tensor slot
        # slots: 0..3 = KT(re_h, re_l, im_h, im_l) [D, ML]; 4,5 = V(re, im) [ML, D]
        self.NSLOT = 6


def build(cfg: Cfg, reps: int = 1, no_collective: bool = False, stop_after: str | None = None):
    c = cfg
    nc = bacc.Bacc("TRN2", target_bir_lowering=False, debug=False, num_devices=c.NC)

    def din(name, shape, dt=FP16):
        return nc.dram_tensor(name, shape, dt, kind="ExternalInput")

    # stationary G = BETA*conj(W_Q)@W_K^T (host-computed, hi/lo fp16,
    # column-sliced per output tile at load time), host-swizzled to
    # [out_block, partition, in_tile*cols] so every per-output-tile slice
    # is one fully-contiguous DMA. "sp" holds G_im - G_re (conj-Karatsuba).
    g = {}
    for comp in ("re", "im", "sp"):
        for lvl in ("h", "l"):
            g[comp, lvl] = din(f"g_{comp}_{lvl}", [c.DT, P, c.DT * P])
    wv = {n: din(f"wv_{n}", [c.DCH, P, c.DT * c.DF]) for n in ("re", "im", "s")}

    # moving operands: R^T with hi/lo splits (+re+im sum variant for the
    # Karatsuba decomposition); Y^T hi-only feeds the fp16 V projection
    rt = {}
    for comp in ("re", "im", "s"):
        for lvl in ("h", "l"):
            rt[comp, lvl] = din(f"rt_{comp}_{lvl}", [P, c.DT * c.NL])
    yt = {comp: din(f"yt_{comp}_h", [P, c.DT * c.ML]) for comp in ("re", "im", "s")}
    # local Y^T slice pre-split hi/lo in AllGather slot layout
    # (slots re_h, re_l, im_h, im_l; each [DT, P, ML] flattened)
    ytb = din("ytb", [4 * c.SLOT])

    ident = din("ident", [P, P])

    o_re = nc.dram_tensor("o_re", [c.NL, c.D], FP32, kind="ExternalOutput")
    o_im = nc.dram_tensor("o_im", [c.NL, c.D], FP32, kind="ExternalOutput")

    with tile.TileContext(nc) as tc:
        with (
            tc.tile_pool(name="pers", bufs=1) as pers,
            tc.tile_pool(name="ps", bufs=1, space="PSUM") as ps,
            tc.tile_pool(name="dram", bufs=1, space="DRAM") as dram,
        ):
            def emit(rep):
                prp = tc.alloc_tile_pool(name=f"prp{rep}", bufs=1)
                kvp = tc.alloc_tile_pool(name=f"kvp{rep}", bufs=1)
                qrt = tc.alloc_tile_pool(name=f"qrt{rep}", bufs=1)
                # ---------- load Y^T (moving operand for K and V projections)
                _scope_stack = []

                def scope(name):
                    if _scope_stack:
                        pn, pid = _scope_stack.pop()
                        nc.leave_named_scope(pn, pid, False)
                    if name is not None:
                        sid, _ = nc.enter_named_scope(name, False)
                        _scope_stack.append((name, sid))

                # collective bounce first: agk_in <- ytb (DRAM->DRAM, no
                # compute dependency) so the Y^T AllGather starts immediately
                agk_in = dram.tile([4 * c.SLOT], FP16)
                agk_out = dram.tile([c.NC * 4 * c.SLOT], FP16, addr_space="Shared")
                scope("ybounce")
                nc.gpsimd.dma_start(agk_in[:], ytb.ap())
                scope("agk")
                if not no_collective:
                    nc.gpsimd.collective_compute(
                        "AllGather",
                        mybir.AluOpType.bypass,
                        replica_groups=[list(range(c.NC))],
                        ins=[agk_in.opt()],
                        outs=[agk_out.opt()],
                    )

                scope("yload")
                yts = {}
                for comp, t in yt.items():
                    yts[comp] = kvp.tile([P, c.DT * c.ML], FP16, tag=f"yt{comp}", name=f"yt_{comp}_{rep}")
                    nc.scalar.dma_start(yts[comp][:], t.ap())

                ident_sb = pers.tile([P, P], FP16, tag="ident")
                nc.sync.dma_start(ident_sb[:], ident.ap())

                # R^T loads hoisted here: they overlap the K projection, so
                # Q's moving operands are resident before qproj starts.
                scope("rload")
                rts = {}
                for key, t in rt.items():
                    rts[key] = qrt.tile([P, c.DT * c.NL], FP16, tag=f"rt{key}", name=f"rt_{key[0]}_{key[1]}_{rep}")
                    nc.scalar.dma_start(rts[key][:], t.ap())

                agv_in = dram.tile([2 * c.SLOT], FP16)
                agv_out = dram.tile([c.NC * 2 * c.SLOT], FP16, addr_space="Shared")

                # ---------- T^T projection: T = conj(R) @ G  (3-pass split)
                # conj-Karatsuba: m1 = Rre@Gre, m2 = Rim@Gim, m3 = Rs@(Gim-Gre);
                # T_re = m1 + m2, T_im' = -Im(T) = m2 - m1 - m3.
                scope("tproj")
                qp = tc.alloc_tile_pool(name=f"qp{rep}", bufs=1)
                qt_sb = {}
                for comp in ("re", "im"):
                    for lvl in ("h", "l"):
                        qt_sb[comp, lvl] = prp.tile([P, c.DT * c.NL], FP16, tag=f"qt{comp}{lvl}", name=f"qt_{comp}_{lvl}_{rep}")
                mw = c.NL
                for dt_out in range(c.DT):
                    wsl = qp.tile([P, 6 * c.DT * P], FP16, tag="wqsl", bufs=2)
                    widx = {("re", "h"): 0, ("re", "l"): 1, ("im", "h"): 2,
                            ("im", "l"): 3, ("sp", "h"): 4, ("sp", "l"): 5}
                    for (wc, wl), wi in widx.items():
                        nc.sync.dma_start(
                            wsl[:, wi * c.DT * P : (wi + 1) * c.DT * P],
                            g[wc, wl].ap()[dt_out],
                        )

                    def wslice(wc, wl, ki):
                        wi = widx[wc, wl]
                        return wsl[:, wi * c.DT * P + ki * P : wi * c.DT * P + (ki + 1) * P]

                    m = {}
                    for prod, (wc, mc) in enumerate(
                        [("re", "re"), ("im", "im"), ("sp", "s")]
                    ):
                        pt = ps.tile([P, 512], FP32, tag="ps", bufs=6)
                        m[prod] = pt[:, :mw]
                        nmm = c.DT * 3
                        i = 0
                        for ki in range(c.DT):
                            for wl, ml in (("h", "h"), ("h", "l"), ("l", "h")):
                                nc.tensor.matmul(
                                    m[prod],
                                    wslice(wc, wl, ki),
                                    rts[mc, ml][:, ki * mw : ki * mw + mw],
                                    start=(i == 0),
                                    stop=(i == nmm - 1),
                                )
                                i += 1
                    # DVE may read only ONE operand from PSUM per inst:
                    # stage m2 in SBUF, then chain single-PSUM ops.
                    m2s = qp.tile([P, 512], FP32, tag="wqm2s", bufs=2)
                    nc.vector.tensor_copy(m2s[:, :mw], m[1])
                    dre = qp.tile([P, 512], FP32, tag="wqdre", bufs=2)
                    nc.vector.tensor_add(dre[:, :mw], m[0], m2s[:, :mw])
                    dim = qp.tile([P, 512], FP32, tag="wqdim", bufs=2)
                    nc.vector.tensor_sub(dim[:, :mw], m2s[:, :mw], m[0])
                    nc.vector.tensor_sub(dim[:, :mw], dim[:, :mw], m[2])
                    for comp, d in (("re", dre), ("im", dim)):
                        hi = qt_sb[comp, "h"][:, dt_out * mw : (dt_out + 1) * mw]
                        lo = qt_sb[comp, "l"][:, dt_out * mw : (dt_out + 1) * mw]
                        nc.vector.tensor_copy(hi, d[:, :mw])
                        nc.vector.tensor_sub(lo, d[:, :mw], hi)
                qp.release()
                qrt.release()
                if stop_after == "qproj":
                    prp.release()
                    scope(None)
                    return

                vp = tc.alloc_tile_pool(name=f"vp{rep}", bufs=1)
                scope("vproj")
                # ---------- V projection (single-pass fp16, Karatsuba):
                # m1 = Yre@WVre, m2 = Yim@WVim, m3 = (Yre+Yim)@(WVre+WVim)
                # V_re = m1 - m2 ; V_im = m3 - m1 - m2
                for dch in range(c.DCH):
                    wvsl = vp.tile([P, 3 * c.DT * c.DF], FP16, tag="wvsl", bufs=2)
                    wvidx = {"re": 0, "im": 1, "s": 2}
                    for wn, wi in wvidx.items():
                        nc.sync.dma_start(
                            wvsl[:, wi * c.DT * c.DF : (wi + 1) * c.DT * c.DF],
                            wv[wn].ap()[dch],
                        )
                    for mt in range(c.MTS):
                        m = {}
                        for prod, yc in enumerate(("re", "im", "s")):
                            pt = ps.tile([P, 512], FP32, tag="ps", bufs=6)
                            m[prod] = pt[:, : c.DF]
                            wn = yc
                            for ki in range(c.DT):
                                nc.tensor.matmul(
                                    m[prod],
                                    yts[yc][:, ki * c.ML + mt * P : ki * c.ML + (mt + 1) * P],
                                    wvsl[:, wvidx[wn] * c.DT * c.DF + ki * c.DF : wvidx[wn] * c.DT * c.DF + (ki + 1) * c.DF],
                                    start=(ki == 0),
                                    stop=(ki == c.DT - 1),
                                )
                        vm2s = vp.tile([P, c.DF], FP32, tag="vm2s", bufs=2)
                        nc.vector.tensor_copy(vm2s[:], m[1])
                        for comp, si in (("re", 0), ("im", 1)):
                            vout = vp.tile([P, c.DF], FP16, tag="vout", bufs=4)
                            if comp == "re":
                                nc.vector.tensor_sub(vout[:], m[0], vm2s[:])
                            else:
                                vim1 = vp.tile([P, c.DF], FP32, tag="vim1", bufs=2)
                                nc.vector.tensor_sub(vim1[:], m[2], vm2s[:])
                                nc.vector.tensor_sub(vout[:], vim1[:], m[0])
                            dst = agv_in[si * c.SLOT : (si + 1) * c.SLOT].rearrange(
                                "(m p dc d) -> m p dc d", m=c.MTS, p=P, dc=c.DCH
                            )[mt, :, dch, :]
                            nc.gpsimd.dma_start(dst, vout[:])

                # ---------- AllGather V (A@V consumes it much later)
                scope("agv")
                if not no_collective:
                    nc.gpsimd.collective_compute(
                        "AllGather",
                        mybir.AluOpType.bypass,
                        replica_groups=[list(range(c.NC))],
                        ins=[agv_in.opt()],
                        outs=[agv_out.opt()],
                    )
                if stop_after == "vproj":
                    vp.release()
                    kvp.release()
                    prp.release()
                    scope(None)
                    return

                vp.release()
                kvp.release()


                # ---------- scores + streaming softmax (per key-shard chunk)
                # Z[q, m] = QT^T @ KT ; chunk max -> exp(Z - cmax); rescale later.
                scope("scores")
                scp = tc.alloc_tile_pool(name=f"scp{rep}", bufs=1)
                p_sb = [prp.tile([P, c.M], FP16, tag=f"p{qt}", name=f"p_{qt}_{rep}") for qt in range(c.QTS)]
                cm = [prp.tile([P, c.NC], FP32, tag=f"cm{qt}", name=f"cm_{qt}_{rep}") for qt in range(c.QTS)]
                ncm = [prp.tile([P, c.NC], FP32, tag=f"ncm{qt}", name=f"ncm_{qt}_{rep}") for qt in range(c.QTS)]

                kdh = c.DT // c.KHALF  # d-tiles per streamed half
                for r in range(c.NC):
                    halves = []
                    for h in range(c.KHALF):
                        ktl = scp.tile([P, 4 * kdh * c.ML], FP16, tag="ktl", bufs=3)
                        for si in range(4):
                            src = agk_out[
                                r * 4 * c.SLOT
                                + si * c.SLOT
                                + h * kdh * P * c.ML : r * 4 * c.SLOT
                                + si * c.SLOT
                                + (h + 1) * kdh * P * c.ML
                            ].rearrange("(t p m) -> p t m", p=P, m=c.ML)
                            nc.scalar.dma_start(
                                ktl[:, si * kdh * c.ML : (si + 1) * kdh * c.ML].rearrange(
                                    "p (t m) -> p t m", m=c.ML
                                ),
                                src,
                            )
                        halves.append(ktl)

                    def ktslice(comp, lvl, ki):
                        si = {("re", "h"): 0, ("re", "l"): 1, ("im", "h"): 2, ("im", "l"): 3}[comp, lvl]
                        t = halves[ki // kdh]
                        k = ki % kdh
                        return t[:, si * kdh * c.ML + k * c.ML : si * kdh * c.ML + (k + 1) * c.ML]

                    for qt in range(c.QTS):
                        zp = ps.tile([P, 512], FP32, tag="ps", bufs=6)
                        zacc = zp[:, : c.ML]
                        nmm = 2 * c.DT * 3
                        i = 0
                        for comp in ("re", "im"):
                            for ki in range(c.DT):
                                for ql, kl in (("h", "h"), ("h", "l"), ("l", "h")):
                                    nc.tensor.matmul(
                                        zacc,
                                        qt_sb[comp, ql][:, ki * c.NL + qt * P : ki * c.NL + (qt + 1) * P],
                                        ktslice(comp, kl, ki),
                                        start=(i == 0),
                                        stop=(i == nmm - 1),
                                    )
                                    i += 1
                        nc.vector.reduce_max(cm[qt][:, r : r + 1], zacc, axis=X)
                        nc.vector.tensor_scalar_mul(
                            ncm[qt][:, r : r + 1], cm[qt][:, r : r + 1], -1.0
                        )
                        nc.scalar.activation(
                            p_sb[qt][:, r * c.ML : (r + 1) * c.ML],
                            zacc,
                            mybir.ActivationFunctionType.Exp,
                            bias=ncm[qt][:, r : r + 1],
                            scale=1.0,
                        )

                # ---------- finalize softmax: rescale chunks to the global max
                scope("smax")
                recip = []
                for qt in range(c.QTS):
                    ngm = prp.tile([P, 1], FP32, tag=f"ngm{qt}")
                    nc.vector.tensor_reduce(
                        ngm[:], ncm[qt][:], op=mybir.AluOpType.min, axis=X
                    )
                    fac = prp.tile([P, c.NC], FP32, tag=f"fac{qt}")
                    nc.scalar.activation(
                        fac[:],
                        ncm[qt][:],
                        mybir.ActivationFunctionType.Exp,
                        bias=ngm[:, 0:1],
                        scale=-1.0,
                    )
                    for r in range(c.NC):
                        nc.vector.tensor_scalar_mul(
                            p_sb[qt][:, r * c.ML : (r + 1) * c.ML],
                            p_sb[qt][:, r * c.ML : (r + 1) * c.ML],
                            fac[:, r : r + 1],
                        )
                    ssum = prp.tile([P, 1], FP32, tag=f"ssum{qt}")
                    nc.vector.reduce_sum(ssum[:], p_sb[qt][:], axis=X)
                    rc = prp.tile([P, 1], FP32, tag=f"rcp{qt}")
                    nc.vector.reciprocal(rc[:], ssum[:])
                    recip.append(rc)

                scp.release()
                if stop_after == "scores":
                    prp.release()
                    scope(None)
                    return

                # ---------- transpose P -> P^T tiles ([m-part, q-free])
                scope("transp")
                avp = tc.alloc_tile_pool(name=f"avp{rep}", bufs=1)
                pt_sb = [avp.tile([P, c.NL], FP16, tag=f"pt{mtg}", name=f"pt_{mtg}_{rep}") for mtg in range(c.MTG)]
                for mtg in range(c.MTG):
                    tp = ps.tile([P, 512], FP16, tag="dsc", bufs=2)
                    tacc = tp[:, : c.NL]
                    for qt in range(c.QTS):
                        nc.tensor.matmul(
                            tacc[:, qt * P : (qt + 1) * P],
                            p_sb[qt][:, mtg * P : (mtg + 1) * P],
                            ident_sb[:],
                            start=True,
                            stop=True,
                            is_transpose=True,
                        )
                    nc.vector.tensor_copy(pt_sb[mtg][:], tacc)
                if stop_after == "transp":
                    avp.release()
                    prp.release()
                    scope(None)
                    return

                # ---------- A @ V (+ 1/sum scaling)
                scope("av")
                for comp, odram in (("re", o_re), ("im", o_im)):
                    si = 0 if comp == "re" else 1
                    for dch in range(c.DCH):
                        vh = avp.tile([P, c.MTG * c.DF], FP16, tag="vh", bufs=2)
                        for r in range(c.NC):
                            src = agv_out[
                                r * 2 * c.SLOT + si * c.SLOT : r * 2 * c.SLOT + (si + 1) * c.SLOT
                            ].rearrange("(m p dc d) -> dc p m d", m=c.MTS, p=P, dc=c.DCH)[dch]
                            nc.sync.dma_start(
                                vh[
                                    :, r * c.MTS * c.DF : (r + 1) * c.MTS * c.DF
                                ].rearrange("p (m d) -> p m d", m=c.MTS),
                                src,
                            )
                        for qt in range(c.QTS):
                            op_ = ps.tile([P, 512], FP32, tag="ps", bufs=6)
                            oacc = op_[:, : c.DF]
                            for mtg in range(c.MTG):
                                nc.tensor.matmul(
                                    oacc,
                                    pt_sb[mtg][:, qt * P : (qt + 1) * P],
                                    vh[:, mtg * c.DF : (mtg + 1) * c.DF],
                                    start=(mtg == 0),
                                    stop=(mtg == c.MTG - 1),
                                )
                            osb = avp.tile([P, c.DF], FP32, tag="osb", bufs=4)
                            nc.vector.tensor_scalar_mul(osb[:], oacc, recip[qt][:, 0:1])
                            nc.sync.dma_start(
                                odram.ap()[
                                    qt * P : (qt + 1) * P, dch * c.DF : (dch + 1) * c.DF
                                ],
                                osb[:],
                            )
                avp.release()
                prp.release()
                scope(None)

            for rep in range(reps):
                emit(rep)

    nc.compile()
    return nc


def _split16(x):
    h = x.astype(np.float16)
    l = (x - h.astype(np.float32)).astype(np.float16)
    return h, l


def prep_inputs(cfg, R_re, R_im, Y_re, Y_im, W_Q_re, W_Q_im, W_K_re, W_K_im, W_V_re, W_V_im):
    """Host-side sharding + fp16 hi/lo split + transposes + G. Returns in_maps."""
    c = cfg
    f32 = np.float32
    f64 = np.float64
    # G = BETA * conj(W_Q) @ W_K^T in float64 (exact to fp16-split precision)
    WQ = np.asarray(W_Q_re, dtype=f64) + 1j * np.asarray(W_Q_im, dtype=f64)
    WK = np.asarray(W_K_re, dtype=f64) + 1j * np.asarray(W_K_im, dtype=f64)
    G = BETA * (np.conj(WQ) @ WK.T)
    g_re, g_im = G.real, G.imag
    gs = {"re": _split16(g_re), "im": _split16(g_im), "sp": _split16(g_im - g_re)}
    wv_re = np.ascontiguousarray(W_V_re, dtype=f32)
    wv_im = np.ascontiguousarray(W_V_im, dtype=f32)
    ident = np.eye(P, dtype=np.float16)

    DT, DCH, DF = cfg.DT, cfg.DCH, cfg.DF

    def _wsw(w16, ocols):
        # [d_in, d_out] -> [d_out_block, p, d_in_tile * ocols], contiguous
        ob = w16.shape[1] // ocols
        return np.ascontiguousarray(
            w16.reshape(DT, P, ob, ocols).transpose(2, 1, 0, 3).reshape(ob, P, DT * ocols)
        )

    shared = {}
    for comp in ("re", "im", "sp"):
        for li, lvl in enumerate(("h", "l")):
            shared[f"g_{comp}_{lvl}"] = _wsw(gs[comp][li], P)
    shared["wv_re"] = _wsw(wv_re.astype(np.float16), DF)
    shared["wv_im"] = _wsw(wv_im.astype(np.float16), DF)
    shared["wv_s"] = _wsw((wv_re + wv_im).astype(np.float16), DF)
    shared["ident"] = ident

    in_maps = []
    for r in range(c.NC):
        m = dict(shared)
        rsl = slice(r * c.NL, (r + 1) * c.NL)
        ysl = slice(r * c.ML, (r + 1) * c.ML)
        rre_t = np.ascontiguousarray(np.asarray(R_re[rsl], dtype=f32).T)
        rim_t = np.ascontiguousarray(np.asarray(R_im[rsl], dtype=f32).T)
        yre_t = np.ascontiguousarray(np.asarray(Y_re[ysl], dtype=f32).T)
        yim_t = np.ascontiguousarray(np.asarray(Y_im[ysl], dtype=f32).T)
        # R^T hi/lo (+sum) in [p, (t m)] layout for the T projection
        for base, arr in (("rt_re", rre_t), ("rt_im", rim_t), ("rt_s", rre_t + rim_t)):
            h, l = _split16(arr)
            mw = arr.shape[1]
            for lvl, a in (("h", h), ("l", l)):
                m[f"{base}_{lvl}"] = np.ascontiguousarray(
                    a.reshape(DT, P, mw).transpose(1, 0, 2).reshape(P, DT * mw)
                )
        # Y^T hi-only (+sum) in [p, (t m)] layout for the fp16 V projection
        for base, arr in (("yt_re", yre_t), ("yt_im", yim_t), ("yt_s", yre_t + yim_t)):
            mw = arr.shape[1]
            m[base + "_h"] = np.ascontiguousarray(
                arr.astype(np.float16).reshape(DT, P, mw).transpose(1, 0, 2).reshape(P, DT * mw)
            )
        # Y^T hi/lo in AllGather slot layout [(t p m)] (slots: re_h re_l im_h im_l)
        yre_h, yre_l = _split16(yre_t)
        yim_h, yim_l = _split16(yim_t)
        m["ytb"] = np.concatenate(
            [a.reshape(-1) for a in (yre_h, yre_l, yim_h, yim_l)]
        )
        in_maps.append(m)
    return in_maps


_NC_CACHE = {}


def kernel(**inputs) -> np.ndarray:
    cfg = Cfg()
    if "full" not in _NC_CACHE:
        _NC_CACHE["full"] = build(cfg, 1)
    nc = _NC_CACHE["full"]
    in_maps = prep_inputs(cfg, **inputs)
    res = run_bass_kernel_spmd(nc, in_maps, list(range(cfg.NC)))
    o_re = np.concatenate([res.results[r]["o_re"] for r in range(cfg.NC)], axis=0)
    o_im = np.concatenate([res.results[r]["o_im"] for r in range(cfg.NC)], axis=0)
    return (o_re + 1j * o_im).astype(np.complex64)



# revision 41
# speedup vs baseline: 59.5181x; 1.1022x over previous
"""Chopfield attention (complex QKV projections + real-part softmax attention)
on 8 Trainium2 NeuronCores.

Math (reference):
    Q = R @ W_Q ; K = Y @ W_K ; V = Y @ W_V          (complex, [4096,1024])
    Z = BETA * Re(conj(Q) @ K^T)                      [4096,4096] real
    A = softmax(Z, axis=-1)                           real
    out = A @ V                                       (complex)

Sharding: queries (R rows) and keys (Y rows) are both sharded 8-way.

Two re-associations remove all compute->collective dependencies:
  G-trick:  Z = Re(conj(R) @ G @ Y^T),  G = BETA * conj(W_Q) @ W_K^T
            precomputed on the HOST (weights only). The device never
            materializes K; it AllGathers raw Y^T instead.
  AY-trick: out = A @ (Y @ W_V) = (A @ Y) @ W_V. The device never
            materializes V; it AllGathers raw row-major Y instead, and
            multiplies by W_V after the attention average.
Both AllGather payloads are raw inputs (host pre-swizzled, bounced
DRAM->DRAM at rep start), so the collectives run entirely under compute.

Per rep: T = conj(R) @ G (3-pass fp16 hi/lo Karatsuba) -> scores
Z = T_re @ Y_re^T + T_im' @ Y_im^T (3-pass hi/lo per product, fp32 PSUM
accumulate) -> streaming softmax -> A^T (PE transpose) -> U^T = Y^T A^T
(fp16) -> out = U @ W_V (fp16 Karatsuba, scaled by 1/rowsum).

Precision: the softmax is near-one-hot (score std ~2900), so the score
chain must be fp32-accurate: G is computed in float64 on host and split
hi/lo fp16; score-chain matmuls use a 3-pass fp16 hi/lo split (fp16
products are exact on the PE and accumulate in fp32), landing within
~2e-3 of a pure-fp32 pipeline. The A/V path tolerates plain fp16.

Software pipelining: rep N's attention-average (transp/U/UW) is emitted
after rep N+1's scores, so the PE never idles during rep N's softmax
epilogue; per-rep tile pools alternate heap sides to satisfy the pool
stack discipline.
"""

import numpy as np

import concourse.bacc as bacc
import concourse.mybir as mybir
import concourse.tile as tile
from concourse.bass_utils import run_bass_kernel_spmd

BETA = 0.03125
P = 128
FP16 = mybir.dt.float16
FP32 = mybir.dt.float32
X = mybir.AxisListType.X


class Cfg:
    def __init__(self, N=4096, M=4096, D=1024, NC=8):
        self.N, self.M, self.D, self.NC = N, M, D, NC
        self.NL = N // NC          # local query rows
        self.ML = M // NC          # local key rows
        self.DT = D // P           # contraction tiles
        self.QTS = self.NL // P    # local query partition-tiles
        self.MTS = self.ML // P    # local key partition-tiles
        self.DF = min(512, D)      # free-dim chunk for D-wide outputs
        self.DCH = D // self.DF    # chunks of D
        self.MTG = M // P          # global key partition-tiles
        self.KHALF = 2 if self.DT % 2 == 0 else 1   # score K-stream halves
        self.SLOT = D * self.ML    # elements per gathered tensor slot
        # agk slots: re_h re_l im_h im_l of Y^T, layout [half][p][k][m]
        # agyr slots: re im of row-major Y, layout [dblock][p][mts][128]


def build(cfg: Cfg, reps: int = 1, no_collective: bool = False):
    c = cfg
    nc = bacc.Bacc("TRN2", target_bir_lowering=False, debug=False, num_devices=c.NC)

    def din(name, shape, dt=FP16):
        return nc.dram_tensor(name, shape, dt, kind="ExternalInput")

    # stationary G = BETA*conj(W_Q)@W_K^T (host fp64, hi/lo fp16 split),
    # host-swizzled to [out_block, partition, in_tile*cols] so every
    # per-output-tile slice is one fully-contiguous DMA.
    # "sp" holds G_im - G_re (conj-Karatsuba third product).
    g = {}
    for comp in ("re", "im", "sp"):
        for lvl in ("h", "l"):
            g[comp, lvl] = din(f"g_{comp}_{lvl}", [c.DT, P, c.DT * P])
    wv = {n: din(f"wv_{n}", [c.DCH, P, c.DT * c.DF]) for n in ("re", "im", "s")}

    # moving operand: R^T with hi/lo splits (+re+im sum variant)
    rt = {}
    for comp in ("re", "im", "s"):
        for lvl in ("h", "l"):
            rt[comp, lvl] = din(f"rt_{comp}_{lvl}", [P, c.DT * c.NL])
    # local Y^T hi/lo in AllGather slot layout (scores stream)
    ytb = din("ytb", [4 * c.SLOT])
    # local row-major Y in AllGather slot layout (attention-average stream)
    ytr = din("ytr", [2 * c.SLOT])

    ident = din("ident", [P, P])

    o_re = nc.dram_tensor("o_re", [c.NL, c.D], FP32, kind="ExternalOutput")
    o_im = nc.dram_tensor("o_im", [c.NL, c.D], FP32, kind="ExternalOutput")

    with tile.TileContext(nc) as tc:
        with (
            tc.tile_pool(name="pers", bufs=1) as pers,
            tc.tile_pool(name="ps", bufs=1, space="PSUM") as ps,
            tc.tile_pool(name="dram", bufs=1, space="DRAM") as dram,
        ):
            ident_sb = pers.tile([P, P], FP16, tag="ident")
            nc.sync.dma_start(ident_sb[:], ident.ap())

            def mkscope():
                stack = []

                def scope(name):
                    if stack:
                        pn, pid = stack.pop()
                        nc.leave_named_scope(pn, pid, False)
                    if name is not None:
                        sid, _ = nc.enter_named_scope(name, False)
                        stack.append((name, sid))

                return scope

            def emit_front(rep):
                side = "left" if rep % 2 == 0 else "right"
                scope = mkscope()
                prp = tc.alloc_tile_pool(name=f"prp{rep}", bufs=1, side=side)
                qtp = tc.alloc_tile_pool(name=f"qtp{rep}", bufs=1, side=side)

                # ---------- bounce both gather payloads, trigger collectives
                agk_in = dram.tile([4 * c.SLOT], FP16)
                agk_out = dram.tile([c.NC * 4 * c.SLOT], FP16, addr_space="Shared")
                agy_in = dram.tile([2 * c.SLOT], FP16)
                agy_out = dram.tile([c.NC * 2 * c.SLOT], FP16, addr_space="Shared")
                scope("ybounce")
                nc.gpsimd.dma_start(agk_in[:], ytb.ap())
                nc.gpsimd.dma_start(agy_in[:], ytr.ap())
                scope("agk")
                if not no_collective:
                    nc.gpsimd.collective_compute(
                        "AllGather",
                        mybir.AluOpType.bypass,
                        replica_groups=[list(range(c.NC))],
                        ins=[agk_in.opt()],
                        outs=[agk_out.opt()],
                    )
                    nc.gpsimd.collective_compute(
                        "AllGather",
                        mybir.AluOpType.bypass,
                        replica_groups=[list(range(c.NC))],
                        ins=[agy_in.opt()],
                        outs=[agy_out.opt()],
                    )

                # ---------- R^T loads (sync ring; overlaps previous rep)
                scope("rload")
                qrt = tc.alloc_tile_pool(name=f"qrt{rep}", bufs=1)
                rts = {}
                for key, t in rt.items():
                    rts[key] = qrt.tile([P, c.DT * c.NL], FP16, tag=f"rt{key}", name=f"rt_{key[0]}_{key[1]}_{rep}")
                    nc.sync.dma_start(rts[key][:], t.ap())

                # ---------- T^T projection: T = conj(R) @ G  (3-pass split)
                # m1 = Rre@Gre, m2 = Rim@Gim, m3 = Rs@(Gim-Gre);
                # T_re = m1 + m2, T_im' = -Im(T) = m2 - m1 - m3.
                scope("tproj")
                qp = tc.alloc_tile_pool(name=f"qp{rep}", bufs=1)
                qt_sb = {}
                for comp in ("re", "im"):
                    for lvl in ("h", "l"):
                        qt_sb[comp, lvl] = qtp.tile([P, c.DT * c.NL], FP16, tag=f"qt{comp}{lvl}", name=f"qt_{comp}_{lvl}_{rep}")
                mw = c.NL
                for dt_out in range(c.DT):
                    wsl = qp.tile([P, 6 * c.DT * P], FP16, tag="wqsl", bufs=3)
                    widx = {("re", "h"): 0, ("re", "l"): 1, ("im", "h"): 2,
                            ("im", "l"): 3, ("sp", "h"): 4, ("sp", "l"): 5}
                    for (wc, wl), wi in widx.items():
                        nc.sync.dma_start(
                            wsl[:, wi * c.DT * P : (wi + 1) * c.DT * P],
                            g[wc, wl].ap()[dt_out],
                        )

                    def wslice(wc, wl, ki):
                        wi = widx[wc, wl]
                        return wsl[:, wi * c.DT * P + ki * P : wi * c.DT * P + (ki + 1) * P]

                    m = {}
                    for prod, (wc, mc) in enumerate(
                        [("re", "re"), ("im", "im"), ("sp", "s")]
                    ):
                        pt = ps.tile([P, 512], FP32, tag="ps", bufs=6)
                        m[prod] = pt[:, :mw]
                        nmm = c.DT * 3
                        i = 0
                        for ki in range(c.DT):
                            for wl, ml in (("h", "h"), ("h", "l"), ("l", "h")):
                                nc.tensor.matmul(
                                    m[prod],
                                    wslice(wc, wl, ki),
                                    rts[mc, ml][:, ki * mw : ki * mw + mw],
                                    start=(i == 0),
                                    stop=(i == nmm - 1),
                                )
                                i += 1
                    # DVE may read only ONE operand from PSUM per inst:
                    # stage m2 in SBUF, then chain single-PSUM ops.
                    m2s = qp.tile([P, 512], FP32, tag="wqm2s", bufs=2)
                    nc.vector.tensor_copy(m2s[:, :mw], m[1])
                    dre = qp.tile([P, 512], FP32, tag="wqdre", bufs=2)
                    nc.vector.tensor_add(dre[:, :mw], m[0], m2s[:, :mw])
                    dim = qp.tile([P, 512], FP32, tag="wqdim", bufs=2)
                    nc.vector.tensor_sub(dim[:, :mw], m2s[:, :mw], m[0])
                    nc.vector.tensor_sub(dim[:, :mw], dim[:, :mw], m[2])
                    for comp, d in (("re", dre), ("im", dim)):
                        hi = qt_sb[comp, "h"][:, dt_out * mw : (dt_out + 1) * mw]
                        lo = qt_sb[comp, "l"][:, dt_out * mw : (dt_out + 1) * mw]
                        nc.vector.tensor_copy(hi, d[:, :mw])
                        nc.vector.tensor_sub(lo, d[:, :mw], hi)
                qp.release()
                qrt.release()
                scope(None)
                return dict(rep=rep, side=side, scope=scope, prp=prp, qtp=qtp,
                            qt_sb=qt_sb, agk_out=agk_out, agy_out=agy_out)

            def emit_front2(st):
                rep, side, scope = st["rep"], st["side"], st["scope"]
                prp, qtp, qt_sb = st["prp"], st["qtp"], st["qt_sb"]
                agk_out = st["agk_out"]

                # ---------- scores + streaming softmax (per key-shard chunk)
                # Z[q, m] = T^T_slices . Y^T ; chunk max -> exp(Z - cmax).
                scope("scores")
                scp = tc.alloc_tile_pool(name=f"scp{rep}", bufs=1)
                p_sb = [prp.tile([P, c.M], FP16, tag=f"p{qt}", name=f"p_{qt}_{rep}") for qt in range(c.QTS)]
                cm = [prp.tile([P, c.NC], FP32, tag=f"cm{qt}", name=f"cm_{qt}_{rep}") for qt in range(c.QTS)]
                ncm = [prp.tile([P, c.NC], FP32, tag=f"ncm{qt}", name=f"ncm_{qt}_{rep}") for qt in range(c.QTS)]

                kdh = c.DT // c.KHALF  # d-tiles per streamed half
                # gathered slot layout is [si][half][p][k][m] (host pre-swizzled)
                # so each (slot, half) load is one contiguous 2D DMA
                for r in range(c.NC):
                    halves = []
                    for h in range(c.KHALF):
                        ktl = scp.tile([P, 4 * kdh * c.ML], FP16, tag="ktl", bufs=4)
                        for si in range(4):
                            base = r * 4 * c.SLOT + si * c.SLOT + h * kdh * P * c.ML
                            src = agk_out[
                                base : base + kdh * P * c.ML
                            ].rearrange("(p km) -> p km", p=P)
                            nc.scalar.dma_start(
                                ktl[:, si * kdh * c.ML : (si + 1) * kdh * c.ML],
                                src,
                            )
                        halves.append(ktl)

                    def ktslice(comp, lvl, ki, halves=halves):
                        si = {("re", "h"): 0, ("re", "l"): 1, ("im", "h"): 2, ("im", "l"): 3}[comp, lvl]
                        t = halves[ki // kdh]
                        k = ki % kdh
                        return t[:, si * kdh * c.ML + k * c.ML : si * kdh * c.ML + (k + 1) * c.ML]

                    for qt in range(c.QTS):
                        zp = ps.tile([P, 512], FP32, tag="ps", bufs=6)
                        zacc = zp[:, : c.ML]
                        nmm = 2 * c.DT * 3
                        i = 0
                        for comp in ("re", "im"):
                            for ki in range(c.DT):
                                for ql, kl in (("h", "h"), ("h", "l"), ("l", "h")):
                                    nc.tensor.matmul(
                                        zacc,
                                        qt_sb[comp, ql][:, ki * c.NL + qt * P : ki * c.NL + (qt + 1) * P],
                                        ktslice(comp, kl, ki),
                                        start=(i == 0),
                                        stop=(i == nmm - 1),
                                    )
                                    i += 1
                        nc.vector.reduce_max(cm[qt][:, r : r + 1], zacc, axis=X)
                        nc.vector.tensor_scalar_mul(
                            ncm[qt][:, r : r + 1], cm[qt][:, r : r + 1], -1.0
                        )
                        nc.scalar.activation(
                            p_sb[qt][:, r * c.ML : (r + 1) * c.ML],
                            zacc,
                            mybir.ActivationFunctionType.Exp,
                            bias=ncm[qt][:, r : r + 1],
                            scale=1.0,
                        )

                # ---------- finalize softmax: rescale chunks to the global max
                scope("smax")
                recip = []
                for qt in range(c.QTS):
                    ngm = prp.tile([P, 1], FP32, tag=f"ngm{qt}")
                    nc.vector.tensor_reduce(
                        ngm[:], ncm[qt][:], op=mybir.AluOpType.min, axis=X
                    )
                    fac = prp.tile([P, c.NC], FP32, tag=f"fac{qt}")
                    nc.scalar.activation(
                        fac[:],
                        ncm[qt][:],
                        mybir.ActivationFunctionType.Exp,
                        bias=ngm[:, 0:1],
                        scale=-1.0,
                    )
                    for r in range(c.NC):
                        nc.vector.tensor_scalar_mul(
                            p_sb[qt][:, r * c.ML : (r + 1) * c.ML],
                            p_sb[qt][:, r * c.ML : (r + 1) * c.ML],
                            fac[:, r : r + 1],
                        )
                    ssum = prp.tile([P, 1], FP32, tag=f"ssum{qt}")
                    nc.vector.reduce_sum(ssum[:], p_sb[qt][:], axis=X)
                    rc = prp.tile([P, 1], FP32, tag=f"rcp{qt}")
                    nc.vector.reciprocal(rc[:], ssum[:])
                    recip.append(rc)

                scp.release()
                qtp.release()
                scope(None)
                st["p_sb"] = p_sb
                st["recip"] = recip

            def emit_back_av(st):
                rep, side, prp = st["rep"], st["side"], st["prp"]
                p_sb, agy_out = st["p_sb"], st["agy_out"]
                scope = mkscope()
                st["bscope"] = scope

                # pool for the UW stage outlives the U stage pool (LIFO)
                avp2 = tc.alloc_tile_pool(name=f"avp2_{rep}", bufs=1, side=side)
                avp = tc.alloc_tile_pool(name=f"avp{rep}", bufs=1, side=side)
                st["avp2"] = avp2

                # ---------- transpose P -> P^T tiles ([m-part, q-free])
                scope("transp")
                pt_sb = [avp.tile([P, c.NL], FP16, tag=f"pt{mtg}", name=f"pt_{mtg}_{rep}") for mtg in range(c.MTG)]
                for mtg in range(c.MTG):
                    tp = ps.tile([P, 512], FP16, tag="dsc", bufs=2)
                    tacc = tp[:, : c.NL]
                    for qt in range(c.QTS):
                        nc.tensor.matmul(
                            tacc[:, qt * P : (qt + 1) * P],
                            p_sb[qt][:, mtg * P : (mtg + 1) * P],
                            ident_sb[:],
                            start=True,
                            stop=True,
                            is_transpose=True,
                        )
                    nc.vector.tensor_copy(pt_sb[mtg][:], tacc)

                # ---------- U^T = Y^T A^T (unnormalized attention average,
                # transposed): stationary = row-Y [m,128d] blocks, moving =
                # A^T tiles; out lands [d-part, q] with no extra transposes.
                scope("av")
                ut = {}
                for comp in ("re", "im"):
                    ut[comp] = avp2.tile([P, c.DT * c.NL], FP16, tag=f"ut{comp}", name=f"ut_{comp}_{rep}")
                # gathered row-Y slot layout [si][db][p][mts][128]; one DMA per
                # (comp, dblock) pulls all ranks: [p, (r mts 128)]
                agy_v = agy_out[:].rearrange(
                    "(r si db p mm) -> si db p r mm", r=c.NC, si=2, db=c.DT, p=P
                )
                for comp, si in (("re", 0), ("im", 1)):
                    for db in range(c.DT):
                        yh = avp.tile([P, c.MTG * P], FP16, tag="yh", bufs=4)
                        nc.sync.dma_start(
                            yh[:].rearrange("p (r mm) -> p r mm", r=c.NC),
                            agy_v[si, db],
                        )
                        up = ps.tile([P, 512], FP32, tag="ps", bufs=6)
                        uacc = up[:, : c.NL]
                        for mtg in range(c.MTG):
                            nc.tensor.matmul(
                                uacc,
                                yh[:, mtg * P : (mtg + 1) * P],
                                pt_sb[mtg][:],
                                start=(mtg == 0),
                                stop=(mtg == c.MTG - 1),
                            )
                        nc.vector.tensor_copy(
                            ut[comp][:, db * c.NL : (db + 1) * c.NL], uacc
                        )
                us = avp2.tile([P, c.DT * c.NL], FP16, tag="uts", name=f"ut_s_{rep}")
                nc.vector.tensor_add(us[:], ut["re"][:], ut["im"][:])
                avp.release()
                scope(None)
                st["ut"] = ut
                st["us"] = us

            def emit_back_uw(st):
                rep, side, prp = st["rep"], st["side"], st["prp"]
                recip, ut, us, avp2 = st["recip"], st["ut"], st["us"], st["avp2"]
                scope = st["bscope"]

                # ---------- out = U @ W_V (fp16 Karatsuba) * 1/rowsum
                # stationary = U^T [d,128q] slices, moving = W_V [d, dout]
                scope("uw")
                wvp = tc.alloc_tile_pool(name=f"wvp{rep}", bufs=1, side=side)
                uts = {"re": ut["re"], "im": ut["im"], "s": us}
                for dch in range(c.DCH):
                    wvsl = wvp.tile([P, 3 * c.DT * c.DF], FP16, tag="wvsl", bufs=2)
                    wvidx = {"re": 0, "im": 1, "s": 2}
                    for wn, wi in wvidx.items():
                        nc.sync.dma_start(
                            wvsl[:, wi * c.DT * c.DF : (wi + 1) * c.DT * c.DF],
                            wv[wn].ap()[dch],
                        )
                    for qt in range(c.QTS):
                        m = {}
                        for prod, un in enumerate(("re", "im", "s")):
                            pt = ps.tile([P, 512], FP32, tag="ps", bufs=6)
                            m[prod] = pt[:, : c.DF]
                            for ki in range(c.DT):
                                nc.tensor.matmul(
                                    m[prod],
                                    uts[un][:, ki * c.NL + qt * P : ki * c.NL + (qt + 1) * P],
                                    wvsl[:, wvidx[un] * c.DT * c.DF + ki * c.DF : wvidx[un] * c.DT * c.DF + (ki + 1) * c.DF],
                                    start=(ki == 0),
                                    stop=(ki == c.DT - 1),
                                )
                        vm2s = avp2.tile([P, c.DF], FP32, tag="vm2s", bufs=2)
                        nc.vector.tensor_copy(vm2s[:], m[1])
                        for comp, odram in (("re", o_re), ("im", o_im)):
                            osb = avp2.tile([P, c.DF], FP32, tag="osb", bufs=4)
                            if comp == "re":
                                nc.vector.tensor_sub(osb[:], m[0], vm2s[:])
                            else:
                                vim1 = avp2.tile([P, c.DF], FP32, tag="vim1", bufs=2)
                                nc.vector.tensor_sub(vim1[:], m[2], vm2s[:])
                                nc.vector.tensor_sub(osb[:], vim1[:], m[0])
                            nc.vector.tensor_scalar_mul(osb[:], osb[:], recip[qt][:, 0:1])
                            nc.sync.dma_start(
                                odram.ap()[
                                    qt * P : (qt + 1) * P, dch * c.DF : (dch + 1) * c.DF
                                ],
                                osb[:],
                            )
                wvp.release()
                avp2.release()
                prp.release()
                scope(None)

            pending = None
            for rep in range(reps):
                st = emit_front(rep)
                if pending is not None:
                    emit_back_av(pending)
                emit_front2(st)
                if pending is not None:
                    emit_back_uw(pending)
                pending = st
            emit_back_av(pending)
            emit_back_uw(pending)

    nc.compile()
    return nc


def _split16(x):
    h = x.astype(np.float16)
    l = (x - h.astype(x.dtype)).astype(np.float16)
    return h, l


def prep_inputs(cfg, R_re, R_im, Y_re, Y_im, W_Q_re, W_Q_im, W_K_re, W_K_im, W_V_re, W_V_im):
    """Host-side sharding + fp16 hi/lo split + transposes + G. Returns in_maps."""
    c = cfg
    f32 = np.float32
    f64 = np.float64
    # G = BETA * conj(W_Q) @ W_K^T in float64 (exact to fp16-split precision)
    WQ = np.asarray(W_Q_re, dtype=f64) + 1j * np.asarray(W_Q_im, dtype=f64)
    WK = np.asarray(W_K_re, dtype=f64) + 1j * np.asarray(W_K_im, dtype=f64)
    G = BETA * (np.conj(WQ) @ WK.T)
    g_re, g_im = G.real, G.imag
    gs = {"re": _split16(g_re), "im": _split16(g_im), "sp": _split16(g_im - g_re)}
    wv_re = np.ascontiguousarray(W_V_re, dtype=f32)
    wv_im = np.ascontiguousarray(W_V_im, dtype=f32)
    ident = np.eye(P, dtype=np.float16)

    DT, DCH, DF = cfg.DT, cfg.DCH, cfg.DF

    def _wsw(w16, ocols):
        # [d_in, d_out] -> [d_out_block, p, d_in_tile * ocols], contiguous
        ob = w16.shape[1] // ocols
        return np.ascontiguousarray(
            w16.reshape(DT, P, ob, ocols).transpose(2, 1, 0, 3).reshape(ob, P, DT * ocols)
        )

    shared = {}
    for comp in ("re", "im", "sp"):
        for li, lvl in enumerate(("h", "l")):
            shared[f"g_{comp}_{lvl}"] = _wsw(gs[comp][li], P)
    shared["wv_re"] = _wsw(wv_re.astype(np.float16), DF)
    shared["wv_im"] = _wsw(wv_im.astype(np.float16), DF)
    shared["wv_s"] = _wsw((wv_re + wv_im).astype(np.float16), DF)
    shared["ident"] = ident

    in_maps = []
    for r in range(c.NC):
        m = dict(shared)
        rsl = slice(r * c.NL, (r + 1) * c.NL)
        ysl = slice(r * c.ML, (r + 1) * c.ML)
        rre_t = np.ascontiguousarray(np.asarray(R_re[rsl], dtype=f32).T)
        rim_t = np.ascontiguousarray(np.asarray(R_im[rsl], dtype=f32).T)
        yre_t = np.ascontiguousarray(np.asarray(Y_re[ysl], dtype=f32).T)
        yim_t = np.ascontiguousarray(np.asarray(Y_im[ysl], dtype=f32).T)
        # R^T hi/lo (+sum) in [p, (t m)] layout for the T projection
        for base, arr in (("rt_re", rre_t), ("rt_im", rim_t), ("rt_s", rre_t + rim_t)):
            h, l = _split16(arr)
            mw = arr.shape[1]
            for lvl, a in (("h", h), ("l", l)):
                m[f"{base}_{lvl}"] = np.ascontiguousarray(
                    a.reshape(DT, P, mw).transpose(1, 0, 2).reshape(P, DT * mw)
                )
        # Y^T hi/lo in AllGather slot layout [si][half][p][k][m]
        # (slots: re_h re_l im_h im_l) so device-side loads are contiguous
        yre_h, yre_l = _split16(yre_t)
        yim_h, yim_l = _split16(yim_t)
        KH = cfg.KHALF
        kdh = DT // KH
        m["ytb"] = np.concatenate([
            np.ascontiguousarray(
                a.reshape(KH, kdh, P, c.ML).transpose(0, 2, 1, 3)
            ).reshape(-1)
            for a in (yre_h, yre_l, yim_h, yim_l)
        ])
        # row-major local Y in slot layout [si][dblock][p][mts][128]
        yre_r = np.asarray(Y_re[ysl], dtype=f32).astype(np.float16)
        yim_r = np.asarray(Y_im[ysl], dtype=f32).astype(np.float16)
        m["ytr"] = np.concatenate([
            np.ascontiguousarray(
                a.reshape(cfg.MTS, P, DT, P).transpose(2, 1, 0, 3)
            ).reshape(-1)
            for a in (yre_r, yim_r)
        ])
        in_maps.append(m)
    return in_maps


_NC_CACHE = {}


def kernel(**inputs) -> np.ndarray:
    cfg = Cfg()
    if "full" not in _NC_CACHE:
        _NC_CACHE["full"] = build(cfg, 1)
    nc = _NC_CACHE["full"]
    in_maps = prep_inputs(cfg, **inputs)
    res = run_bass_kernel_spmd(nc, in_maps, list(range(cfg.NC)))
    o_re = np.concatenate([res.results[r]["o_re"] for r in range(cfg.NC)], axis=0)
    o_im = np.concatenate([res.results[r]["o_im"] for r in range(cfg.NC)], axis=0)
    return (o_re + 1j * o_im).astype(np.complex64)
